# revision 56
# baseline (speedup 1.0000x reference)
"""Trainium2 Bass kernel for nn_MultiHeadAttention_59158879535767.

Reference semantics (B=4, S=2048, D=1024, H=16, DK=64):
  Q = q @ w_q.T + b_q  (same for K, V), reshaped (B,S,DK,H);
  score contracts over the HEAD axis per token: score[t] = Q_t @ K_t.T / 8
  (64x64 per token), softmax over last axis, attn[t] = score @ V_t -> (64,16),
  flattened, then @ w_o.T + b_o.

Everything is per-token => data-parallel over the 8192 tokens across 8 cores,
no collectives. Per core T=1024 tokens, processed in NQ=4 quarters of T4=256.

On-device dataflow per core (all matmuls bf16 with fp32 PSUM accumulation):
  * QKV projections: lhsT = host-permuted W.T tiles [din, (j, hb, d)] where
    output column j*128 + hb*64 + d holds dout = d*16 + (2j+hb). rhs = host-
    transposed x.T [din, tok]. PSUM [128=(hb,d), T4]; bias folded at evac.
    Q/K evac: ONE full-width [128, 128] op per (j, t-half) (rows 0:64 land
    at slot j; rows 64:128 stage at slot j of partitions 64:128 of the same
    tile - uniform AP); per-32-token-chunk SBUF->SBUF DMAs then shift the
    staged odd half down to slots 8:16 (engines cannot move partitions).
    Evac halves alternate Scalar/Vector; weight loads are staggered
    (first j-blocks land early) and x is prefetched one quarter ahead so
    its descriptors beat the shift-DMA flood into the queues.
  * Layouts: QT/KT [128, T4, 32 s] (s = h slot; rows 0:64 data, 16..31
    zeroed; rows 64:128 odd-h staging), VT [128, 17, T4] (slot 16 = ones,
    rows 64:128 duplicate of 0:64 via DMA).
  * S1 per 4-token group: one PE transpose each of QT/KT [64, (tau,s)=128]
    -> pack [128=(tau,s), 64] in PSUM (8 groups batched per bank), evac to
    SBUF. Then per token tau: matmul K=32 rows at base 32*tau:
      lhsT=Kpk[32t:+32, 64g:+64], rhs=Qpk[...] -> ET [64 e, 64 d] at
      (64*(tau%2) partitions, 64*(tau//2) free) of a [128,128] PSUM quarter.
  * exp via ACT on [128, 512] (4 groups) -> E bf16. No max subtraction needed
    (|score| <= ~3 for this distribution).
  * S2 per token: lhsT = E-slice [64 e, 64 d], rhs = VT_eh1d[64q:+64, :, t]
    [64, 17] -> out [64 d, 17] (slot 16 = sum of exp = softmax denominator).
  * normalize: A_norm = A[:, :, 0:16] * recip(A[:, :, 16]) -> bf16, written
    h-major into the all-quarter a_nm [128, 16 h, 512 t] (muls on GpSimd).
  * O-projection: deferred per-(quarter, m) blocks; per h one even-parity
    matmul on PE rows 0:64 and one odd-parity on rows 64:128 (wo rows
    64:128 duplicate 0:64) so every LDWEIGHTS hides under the other
    parity's stream; blocks of quarter q-1 are interleaved between quarter
    q's attention batches as PE stall-filler; + b_o at evac -> out DRAM
    [8 m, 2 par, 128, 512] bf16, host reassembles.
"""
import numpy as np
import ml_dtypes

B, S, D, H, DK = 4, 2048, 1024, 16, 64
NCORE = 8
T = (B * S) // NCORE          # 1024 tokens per core
NQ = 4
T4 = T // NQ                  # 256 tokens per quarter
TP = T4 // 2                  # 128 tokens per parity per quarter
NB = T4 // 32                 # 8 batches of 8 groups (32 tokens) per quarter

bf16 = ml_dtypes.bfloat16

_NC_CACHE = {}


def build_nc():
    import concourse.bacc as bacc
    import concourse.mybir as mybir
    import concourse.tile as tile
    from concourse.masks import make_identity

    nc = bacc.Bacc()
    dt = mybir.dt
    f32, b16 = dt.float32, dt.bfloat16

    # ---- DRAM I/O ----
    xq_d = nc.dram_tensor("xq", [8, 128, T], b16, kind="ExternalInput")
    xk_d = nc.dram_tensor("xk", [8, 128, T], b16, kind="ExternalInput")
    xv_d = nc.dram_tensor("xv", [8, 128, T], b16, kind="ExternalInput")
    wq_d = nc.dram_tensor("wq", [8, 128, 1024], b16, kind="ExternalInput")
    wk_d = nc.dram_tensor("wk", [8, 128, 1024], b16, kind="ExternalInput")
    wv_d = nc.dram_tensor("wv", [8, 128, 1024], b16, kind="ExternalInput")
    wo_d = nc.dram_tensor("wo", [128, 16, 1024], b16, kind="ExternalInput")
    bq_d = nc.dram_tensor("bq", [8, 128], f32, kind="ExternalInput")
    bk_d = nc.dram_tensor("bk", [8, 128], f32, kind="ExternalInput")
    bv_d = nc.dram_tensor("bv", [8, 128], f32, kind="ExternalInput")
    bo_d = nc.dram_tensor("bo", [8, 128], f32, kind="ExternalInput")
    out_d = nc.dram_tensor("out", [8, 2, 128, 4 * TP], b16, kind="ExternalOutput")

    with tile.TileContext(nc) as tc:
        with (
            tc.tile_pool(name="const", bufs=1) as const,
            tc.tile_pool(name="xin", bufs=2) as xin,
            tc.tile_pool(name="work", bufs=2) as work,
            tc.tile_pool(name="epool", bufs=3) as epool,
            tc.tile_pool(name="outp", bufs=2) as outp,
            tc.tile_pool(name="ps", bufs=6, space="PSUM") as ps,
            tc.tile_pool(name="pso", bufs=2, space="PSUM") as pso,
        ):
            # ---- persistent SBUF ----
            wq_s = const.tile([128, 8, 1024], b16, tag="wq")
            wk_s = const.tile([128, 8, 1024], b16, tag="wk")
            wv_s = const.tile([128, 8, 1024], b16, tag="wv")
            wo_s = const.tile([128, 16, 1024], b16, tag="wo")
            bq_s = const.tile([128, 8], f32, tag="bq")
            bk_s = const.tile([128, 8], f32, tag="bk")
            bv_s = const.tile([128, 8], f32, tag="bv")
            bo_s = const.tile([128, 8], f32, tag="bo")
            ident = const.tile([128, 128], b16, tag="ident")
            make_identity(nc, ident)

            # x double-buffers (explicit, so quarter-0 loads can interleave
            # with the weight loads: Q-proj work starts as early as possible)
            xt = {}
            for nm in ("xq", "xk", "xv"):
                for pb in range(2):
                    xt[nm, pb] = xin.tile([128, 8, T4], b16,
                                          tag=f"{nm}{pb}", name=f"{nm}{pb}")

            def load_x(nm, xd, qq):
                tsl = slice(qq * T4, (qq + 1) * T4)
                nc.sync.dma_start(
                    out=xt[nm, qq % 2][:],
                    in_=xd[:, :, tsl].rearrange("ko p t -> p ko t"))

            # staggered weight loads: each tensor's first j-blocks (m 0:256)
            # land early so its projection starts while the rest streams
            load_x("xq", xq_d, 0)
            wq_r = wq_d.rearrange("ko p m -> p ko m")
            wk_r = wk_d.rearrange("ko p m -> p ko m")
            wv_r = wv_d.rearrange("ko p m -> p ko m")
            nc.sync.dma_start(out=wq_s[:, :, 0:256], in_=wq_r[:, :, 0:256])
            nc.sync.dma_start(out=bq_s[:], in_=bq_d.rearrange("j p -> p j"))
            nc.sync.dma_start(out=wq_s[:, :, 256:1024], in_=wq_r[:, :, 256:1024])
            load_x("xk", xk_d, 0)
            nc.sync.dma_start(out=wk_s[:, :, 0:256], in_=wk_r[:, :, 0:256])
            nc.sync.dma_start(out=bk_s[:], in_=bk_d.rearrange("j p -> p j"))
            nc.sync.dma_start(out=wk_s[:, :, 256:1024], in_=wk_r[:, :, 256:1024])
            load_x("xv", xv_d, 0)
            nc.sync.dma_start(out=wv_s[:, :, 0:256], in_=wv_r[:, :, 0:256])
            nc.sync.dma_start(out=bv_s[:], in_=bv_d.rearrange("j p -> p j"))
            nc.sync.dma_start(out=wv_s[:, :, 256:1024], in_=wv_r[:, :, 256:1024])
            # wo/bo are first needed by oproj (mid-quarter-2); loaded inside
            # quarter 0's body so the startup burst isn't bandwidth-starved

            # token-major [64, T4, 32] so the pack-transpose weights AP is
            # contiguous (BIR requires a collapsible stationary AP)
            qt_s = const.tile([128, T4, 32], b16, tag="qt")   # rows 0:64 used
            kt_s = const.tile([128, T4, 32], b16, tag="kt")
            vt_s = const.tile([128, 17, T4], b16, tag="vt")
            # odd-h staging for V (slot-major, rows 64:128 used); Q/K stage
            # their odd half inside qt_s/kt_s rows 64:128 at slot j
            odd_sv = const.tile([128, 8, T4], b16, tag="oddv")
            a_st = const.tile([128, TP, 17], b16, tag="ast")
            zr_s = const.tile([128, TP], f32, tag="zr")
            # h-major, all 4 quarters: col = qq*TP + tp, partition = (par, d)
            a_nm = const.tile([128, 16, 4 * TP], b16, tag="anorm")

            # zero pad slots (s = 16..32) of QT/KT once; ones slot for V once
            nc.any.memset(qt_s[0:64, :, 16:32], 0.0)
            nc.any.memset(kt_s[0:64, :, 16:32], 0.0)
            nc.any.memset(vt_s[0:64, 16, :], 1.0)

            def evac(eng, dst, src, bias):
                if eng == "v":
                    nc.vector.tensor_scalar_add(dst, src, bias)
                else:
                    nc.scalar.activation(
                        dst, src, mybir.ActivationFunctionType.Identity,
                        bias=bias, scale=1.0)

            def oproj_block(qx, m):
                # O-projection m-block over quarter qx (a_nm cols qx*TP..).
                # per h: even-parity matmul on PE rows 0:64, odd on rows
                # 64:128 (wo_s rows 64:128 duplicate 0:64) -> alternating
                # row-groups let each LDWEIGHTS hide under the other matmul's
                # stream.  Own 2-bank PSUM pool so attention batches keep
                # their 6 banks.  One m-block is emitted between consecutive
                # attention batches: the PE chews it while a batch waits for
                # its slot-shift DMA chunk.
                csl = slice(qx * TP, (qx + 1) * TP)
                po_e = pso.tile([128, TP], f32, tag="pso", name="poe")
                po_o = pso.tile([128, TP], f32, tag="pso", name="poo")
                for h in range(16):
                    nc.tensor.matmul(
                        po_e, wo_s[0:64, h, m * 128:(m + 1) * 128],
                        a_nm[0:64, h, csl],
                        start=(h == 0), stop=(h == 15))
                    nc.tensor.matmul(
                        po_o, wo_s[64:128, h, m * 128:(m + 1) * 128],
                        a_nm[64:128, h, csl],
                        start=(h == 0), stop=(h == 15))
                o_e = outp.tile([128, TP], b16, tag="o", name="oe")
                o_o = outp.tile([128, TP], b16, tag="o", name="oo")
                nc.scalar.activation(
                    o_e[:], po_e[:], mybir.ActivationFunctionType.Identity,
                    bias=bo_s[:, m:m + 1], scale=1.0)
                nc.scalar.activation(
                    o_o[:], po_o[:], mybir.ActivationFunctionType.Identity,
                    bias=bo_s[:, m:m + 1], scale=1.0)
                nc.sync.dma_start(out=out_d[m, 0, :, csl], in_=o_e[:])
                nc.sync.dma_start(out=out_d[m, 1, :, csl], in_=o_o[:])

            for qq in range(NQ):
                # prefetch NEXT quarter's x now, so those descriptors enter
                # the DMA queues ahead of this quarter's shift-DMA flood
                if qq + 1 < NQ:
                    load_x("xq", xq_d, qq + 1)
                    load_x("xk", xk_d, qq + 1)
                    load_x("xv", xv_d, qq + 1)
                if qq == 0:
                    nc.sync.dma_start(out=wo_s[:], in_=wo_d[:])
                    nc.sync.dma_start(out=bo_s[:], in_=bo_d.rearrange("j p -> p j"))

                # ---------- projections ----------
                # evac engines: each tensor's even/odd halves go to different
                # engines so Scalar+Vector chew each j-block concurrently;
                # odd-slot partition-shift DMA issued per-j so it pipelines.
                for x_t, w_s, b_s, dst, tmaj, eng_ev, eng_od in (
                    (xt["xq", qq % 2], wq_s, bq_s, qt_s, True, "v", "s"),
                    (xt["xk", qq % 2], wk_s, bk_s, kt_s, True, "s", "v"),
                    (xt["xv", qq % 2], wv_s, bv_s, vt_s, False, "s", "v"),
                ):
                    for j in range(8):
                        pj = ps.tile([128, 512], f32, tag="ps", name="pj")[:, :T4]
                        for ko in range(8):
                            nc.tensor.matmul(
                                pj, w_s[:, ko, j * 128:(j + 1) * 128],
                                x_t[:, ko, :],
                                start=(ko == 0), stop=(ko == 7))
                        if tmaj:
                            # full-width evacs in two t-halves: rows 0:64
                            # (h=2j) land at slot j directly; rows 64:128
                            # (h=2j+1) stage at slot j of partitions 64:128
                            # (uniform AP); shift DMAs fold them to slots
                            # 8:16 in t-chunks so attention batch 0 only
                            # waits for chunk 0
                            evac(eng_ev, dst[:, 0:128, j], pj[:, 0:128],
                                 b_s[:, j:j + 1])
                            evac(eng_od, dst[:, 128:256, j], pj[:, 128:256],
                                 b_s[:, j:j + 1])
                        else:
                            evac(eng_ev, dst[0:64, j, :], pj[0:64, :],
                                 b_s[0:64, j:j + 1])
                            evac(eng_od, odd_sv[64:128, j, :], pj[64:128, :],
                                 b_s[64:128, j:j + 1])
                    if not tmaj:
                        nc.sync.dma_start(
                            out=dst[0:64, 8:16, :], in_=odd_sv[64:128, :, :])
                    elif dst is kt_s:
                        # partition-shifting SBUF->SBUF DMAs for slots 8..16,
                        # one chunk per 32-token attention batch, Q and K
                        # INTERLEAVED in queue order (a batch needs both; Q's
                        # whole set ahead of K's would head-of-line block
                        # K-chunk0 behind 8 Q chunks)
                        for c in range(8):
                            cs = slice(32 * c, 32 * c + 32)
                            nc.sync.dma_start(
                                out=qt_s[0:64, cs, 8:16],
                                in_=qt_s[64:128, cs, 0:8])
                            nc.sync.dma_start(
                                out=kt_s[0:64, cs, 8:16],
                                in_=kt_s[64:128, cs, 0:8])
                # duplicate V rows (incl. ones slot) to partitions 64:128
                nc.sync.dma_start(out=vt_s[64:128, :, :], in_=vt_s[0:64, :, :])



                # ---------- attention ----------
                for b in range(NB):          # 8 batches x 8 groups x 4 tokens
                    qpk_ps = ps.tile([128, 512], b16, tag="ps", name="qpk_ps")
                    kpk_ps = ps.tile([128, 512], b16, tag="ps", name="kpk_ps")
                    for gi in range(8):
                        g = 8 * b + gi
                        for src, pdst in ((qt_s, qpk_ps), (kt_s, kpk_ps)):
                            in_ = src[0:64, 4 * g:4 * g + 4, :]  # [64, 4, 32]
                            nc.tensor.transpose(
                                pdst[:, 64 * gi:64 * gi + 64], in_,
                                ident[0:64, 0:64])
                    qpk = work.tile([128, 512], b16, tag="qpk")
                    kpk = work.tile([128, 512], b16, tag="kpk")
                    nc.vector.tensor_copy(qpk[:], qpk_ps[:])
                    nc.vector.tensor_copy(kpk[:], kpk_ps[:])

                    # each token tau gets a unique (partition-half, PSUM bank):
                    # concurrent matmul drains/clears into the same bank+rows
                    # are a hardware race (observed fatal on device)
                    et_b = [ps.tile([128, 512], f32, tag="ps", name="et0"),
                            ps.tile([128, 512], f32, tag="ps", name="et1")]
                    for gi in range(8):
                        for tau in range(4):
                            nc.tensor.matmul(
                                et_b[tau // 2][64 * (tau % 2):64 * (tau % 2) + 64,
                                               64 * gi:64 * gi + 64],
                                kpk[32 * tau:32 * tau + 32,
                                    64 * gi:64 * gi + 64],
                                qpk[32 * tau:32 * tau + 32,
                                    64 * gi:64 * gi + 64],
                                start=True, stop=True,
                                tile_position=(32 * tau, 64 * (tau % 2)))
                    e_b = [epool.tile([128, 512], b16, tag="e0", name="e0"),
                           epool.tile([128, 512], b16, tag="e1", name="e1")]
                    nc.scalar.activation(e_b[0][:], et_b[0][:],
                                         mybir.ActivationFunctionType.Exp)
                    nc.scalar.activation(e_b[1][:], et_b[1][:],
                                         mybir.ActivationFunctionType.Exp)
                    pa_b = [ps.tile([128, 8, 17], f32, tag="ps", name="pa0"),
                            ps.tile([128, 8, 17], f32, tag="ps", name="pa1")]
                    for gi in range(8):
                        for tau in range(4):
                            t = 32 * b + 4 * gi + tau
                            par = tau % 2
                            nc.tensor.matmul(
                                pa_b[tau // 2][64 * par:64 * par + 64, gi, :],
                                e_b[tau // 2][64 * par:64 * par + 64,
                                              64 * gi:64 * gi + 64],
                                vt_s[64 * par:64 * par + 64, :, t],
                                start=True, stop=True)
                    # tp = 16b + 2gi + tau//2 -> even/odd interleave per bank
                    nc.vector.tensor_copy(
                        a_st[:, 16 * b:16 * b + 16:2, :], pa_b[0][:])
                    nc.vector.tensor_copy(
                        a_st[:, 16 * b + 1:16 * b + 16:2, :], pa_b[1][:])

                    # previous quarter's O-projection m-block: PE filler for
                    # this quarter's next batch's shift-chunk wait
                    if qq > 0:
                        oproj_block(qq - 1, b)

                # ---------- normalize (h-major into the all-quarter a_nm) ----
                nc.vector.reciprocal(zr_s[:], a_st[:, :, 16])
                for h in range(16):
                    nc.gpsimd.tensor_mul(
                        a_nm[:, h, qq * TP:(qq + 1) * TP],
                        a_st[:, :, h], zr_s[:, :])

            # last quarter's O-projection (the only non-overlapped one)
            for m in range(8):
                oproj_block(NQ - 1, m)
    nc.compile()
    return nc


def host_prep(q, k, v, w_q, b_q, w_k, b_k, w_v, b_v, w_o, b_o):
    j = np.arange(8)[:, None, None]
    hb = np.arange(2)[None, :, None]
    d = np.arange(64)[None, None, :]
    perm = (d * 16 + 2 * j + hb).reshape(-1)

    def prep_w(w, scale=1.0):
        wt = (w[perm, :].T.astype(np.float32) * scale).astype(bf16)
        return np.ascontiguousarray(wt.reshape(8, 128, 1024))

    com = dict(
        wq=prep_w(w_q, 0.125), wk=prep_w(w_k), wv=prep_w(w_v),
        bq=np.ascontiguousarray((b_q[perm] * 0.125).reshape(8, 128)).astype(np.float32),
        bk=np.ascontiguousarray(b_k[perm].reshape(8, 128)).astype(np.float32),
        bv=np.ascontiguousarray(b_v[perm].reshape(8, 128)).astype(np.float32),
        bo=np.ascontiguousarray(b_o.reshape(8, 128)).astype(np.float32),
    )
    # V slot order: slot j = h 2j (j<8), slot 8+j = h 2j+1
    hmap = np.array([2 * j for j in range(8)] + [2 * j + 1 for j in range(8)])
    wo_half = np.transpose(w_o.reshape(1024, 64, 16), (1, 2, 0))[:, hmap, :]
    com["wo"] = np.ascontiguousarray(
        np.concatenate([wo_half, wo_half], axis=0).astype(bf16))

    in_maps = []
    for c in range(NCORE):
        m = dict(com)
        for name, x in (("xq", q), ("xk", k), ("xv", v)):
            sl = x.reshape(-1, D)[c * T:(c + 1) * T, :]
            m[name] = np.ascontiguousarray(sl.T.astype(bf16).reshape(8, 128, T))
        in_maps.append(m)
    return in_maps


def reassemble(results):
    # per-core out [8, 2, 128, 512] -> [B, S, D]
    full = np.empty((NCORE, T, D), np.float32)
    for c, res in enumerate(results):
        od = res["out"]                     # [m=8, par=2, p=128, col=512]
        # col = qq*TP + tp ; token t = qq*T4 + 2*tp + par ; D = m*128 + p
        o = np.transpose(od, (3, 1, 0, 2))  # [col, par, m, p]
        full[c] = o.reshape(4, TP, 2, D).reshape(4, T4, D).reshape(T, D)
    return full.reshape(B, S, D)


def kernel(**inputs):
    from concourse.bass_utils import run_bass_kernel_spmd
    if "nc" not in _NC_CACHE:
        _NC_CACHE["nc"] = build_nc()
    nc = _NC_CACHE["nc"]
    in_maps = host_prep(**inputs)
    r = run_bass_kernel_spmd(nc, in_maps, core_ids=list(range(NCORE)))
    return reassemble(r.results)


if __name__ == "__main__":
    z = np.load("/root/problem/inputs_cache.npz")
    inputs = {kk: z[kk] for kk in z.files}
    expd = np.load("/root/problem/expected64.npy")
    act = kernel(**inputs)
    err = np.abs(act - expd)
    scale = np.abs(expd).max()
    print("absmax err:", err.max(), "rel:", err.max() / scale)



# revision 59
# speedup vs baseline: 1.1568x; 1.1568x over previous
"""Trainium2 Bass kernel for nn_MultiHeadAttention_59158879535767.

Reference semantics (B=4, S=2048, D=1024, H=16, DK=64):
  Q = q @ w_q.T + b_q  (same for K, V), reshaped (B,S,DK,H);
  score contracts over the HEAD axis per token: score[t] = Q_t @ K_t.T / 8
  (64x64 per token), softmax over last axis, attn[t] = score @ V_t -> (64,16),
  flattened, then @ w_o.T + b_o.

Everything is per-token => data-parallel over the 8192 tokens across 8 cores,
no collectives. Per core T=1024 tokens, processed in NQ=4 quarters of T4=256.

On-device dataflow per core (all matmuls bf16 with fp32 PSUM accumulation):
  * QKV projections: lhsT = host-permuted W.T tiles [din, (j, hb, d)] where
    output column j*128 + hb*64 + d holds dout = d*16 + (2j+hb). rhs = host-
    transposed x.T [din, tok]. PSUM [128=(hb,d), T4]; bias folded at evac.
    Q/K evac: ONE full-width [128, 128] op per (j, t-half) (rows 0:64 land
    at slot j; rows 64:128 stage at slot j of partitions 64:128 of the same
    tile - uniform AP); per-32-token-chunk SBUF->SBUF DMAs then shift the
    staged odd half down to slots 8:16 (engines cannot move partitions).
    Evac halves alternate Scalar/Vector; weight loads are staggered
    (first j-blocks land early) and x is prefetched one quarter ahead so
    its descriptors beat the shift-DMA flood into the queues.
  * Layouts: QT/KT [128, T4, 32 s] (s = h slot; rows 0:64 data, 16..31
    zeroed; rows 64:128 odd-h staging), VT [128, 17, T4] (slot 16 = ones,
    rows 64:128 duplicate of 0:64 via DMA).
  * S1 per 4-token group: one PE transpose each of QT/KT [64, (tau,s)=128]
    -> pack [128=(tau,s), 64] in PSUM (8 groups batched per bank), evac to
    SBUF. Then per token tau: matmul K=32 rows at base 32*tau:
      lhsT=Kpk[32t:+32, 64g:+64], rhs=Qpk[...] -> ET [64 e, 64 d] at
      (64*(tau%2) partitions, 64*(tau//2) free) of a [128,128] PSUM quarter.
  * exp via ACT on [128, 512] (4 groups) -> E bf16. No max subtraction needed
    (|score| <= ~3 for this distribution).
  * S2 per token: lhsT = E-slice [64 e, 64 d], rhs = VT_eh1d[64q:+64, :, t]
    [64, 17] -> out [64 d, 17] (slot 16 = sum of exp = softmax denominator).
  * normalize: A_norm = A[:, :, 0:16] * recip(A[:, :, 16]) -> bf16, written
    h-major into the all-quarter a_nm [128, 16 h, 512 t] (muls on GpSimd).
  * O-projection: deferred per-(quarter, m) blocks; per h one even-parity
    matmul on PE rows 0:64 and one odd-parity on rows 64:128 (wo rows
    64:128 duplicate 0:64) so every LDWEIGHTS hides under the other
    parity's stream; blocks of quarter q-1 are interleaved between quarter
    q's attention batches as PE stall-filler; + b_o at evac -> out DRAM
    [8 m, 2 par, 128, 512] bf16, host reassembles.
"""
import numpy as np
import ml_dtypes

B, S, D, H, DK = 4, 2048, 1024, 16, 64
NCORE = 8
T = (B * S) // NCORE          # 1024 tokens per core
NQ = 4
T4 = T // NQ                  # 256 tokens per quarter
TP = T4 // 2                  # 128 tokens per parity per quarter
NB = T4 // 32                 # 8 batches of 8 groups (32 tokens) per quarter

bf16 = ml_dtypes.bfloat16

_NC_CACHE = {}


def build_nc():
    import concourse.bacc as bacc
    import concourse.mybir as mybir
    import concourse.tile as tile
    from concourse.masks import make_identity

    nc = bacc.Bacc()
    dt = mybir.dt
    f32, b16 = dt.float32, dt.bfloat16

    # ---- DRAM I/O ----
    xq_d = nc.dram_tensor("xq", [8, 128, T], b16, kind="ExternalInput")
    xk_d = nc.dram_tensor("xk", [8, 128, T], b16, kind="ExternalInput")
    xv_d = nc.dram_tensor("xv", [8, 128, T], b16, kind="ExternalInput")
    wq_d = nc.dram_tensor("wq", [8, 128, 1024], b16, kind="ExternalInput")
    wk_d = nc.dram_tensor("wk", [8, 128, 1024], b16, kind="ExternalInput")
    wv_d = nc.dram_tensor("wv", [8, 128, 1024], b16, kind="ExternalInput")
    wo_d = nc.dram_tensor("wo", [128, 16, 1024], b16, kind="ExternalInput")
    bq_d = nc.dram_tensor("bq", [8, 128], f32, kind="ExternalInput")
    bk_d = nc.dram_tensor("bk", [8, 128], f32, kind="ExternalInput")
    bv_d = nc.dram_tensor("bv", [8, 128], f32, kind="ExternalInput")
    bo_d = nc.dram_tensor("bo", [8, 128], f32, kind="ExternalInput")
    out_d = nc.dram_tensor("out", [8, 2, 128, 4 * TP], b16, kind="ExternalOutput")

    with tile.TileContext(nc) as tc:
        with (
            tc.tile_pool(name="const", bufs=1) as const,
            tc.tile_pool(name="xin", bufs=2) as xin,
            tc.tile_pool(name="work", bufs=2) as work,
            tc.tile_pool(name="epool", bufs=3) as epool,
            tc.tile_pool(name="outp", bufs=2) as outp,
            tc.tile_pool(name="ps", bufs=6, space="PSUM") as ps,
            tc.tile_pool(name="pso", bufs=2, space="PSUM") as pso,
        ):
            # ---- persistent SBUF ----
            wq_s = const.tile([128, 8, 1024], b16, tag="wq")
            wk_s = const.tile([128, 8, 1024], b16, tag="wk")
            wv_s = const.tile([128, 8, 1024], b16, tag="wv")
            wo_s = const.tile([128, 16, 1024], b16, tag="wo")
            bq_s = const.tile([128, 8], f32, tag="bq")
            bk_s = const.tile([128, 8], f32, tag="bk")
            bv_s = const.tile([128, 8], f32, tag="bv")
            bo_s = const.tile([128, 8], f32, tag="bo")
            ident = const.tile([128, 128], b16, tag="ident")
            make_identity(nc, ident)

            # x double-buffers (explicit, so quarter-0 loads can interleave
            # with the weight loads: Q-proj work starts as early as possible)
            xt = {}
            for nm in ("xq", "xk", "xv"):
                for pb in range(2):
                    xt[nm, pb] = xin.tile([128, 8, T4], b16,
                                          tag=f"{nm}{pb}", name=f"{nm}{pb}")

            def load_x(nm, xd, qq):
                tsl = slice(qq * T4, (qq + 1) * T4)
                nc.sync.dma_start(
                    out=xt[nm, qq % 2][:],
                    in_=xd[:, :, tsl].rearrange("ko p t -> p ko t"))

            # staggered weight loads: each tensor's first j-blocks (m 0:256)
            # land early so its projection starts while the rest streams
            load_x("xq", xq_d, 0)
            wq_r = wq_d.rearrange("ko p m -> p ko m")
            wk_r = wk_d.rearrange("ko p m -> p ko m")
            wv_r = wv_d.rearrange("ko p m -> p ko m")
            nc.sync.dma_start(out=wq_s[:, :, 0:256], in_=wq_r[:, :, 0:256])
            nc.sync.dma_start(out=bq_s[:], in_=bq_d.rearrange("j p -> p j"))
            nc.sync.dma_start(out=wq_s[:, :, 256:1024], in_=wq_r[:, :, 256:1024])
            load_x("xk", xk_d, 0)
            nc.sync.dma_start(out=wk_s[:, :, 0:256], in_=wk_r[:, :, 0:256])
            nc.sync.dma_start(out=bk_s[:], in_=bk_d.rearrange("j p -> p j"))
            nc.sync.dma_start(out=wk_s[:, :, 256:1024], in_=wk_r[:, :, 256:1024])
            load_x("xv", xv_d, 0)
            nc.sync.dma_start(out=wv_s[:, :, 0:256], in_=wv_r[:, :, 0:256])
            nc.sync.dma_start(out=bv_s[:], in_=bv_d.rearrange("j p -> p j"))
            nc.sync.dma_start(out=wv_s[:, :, 256:1024], in_=wv_r[:, :, 256:1024])
            # wo/bo are first needed by oproj (mid-quarter-2); loaded inside
            # quarter 0's body so the startup burst isn't bandwidth-starved

            # token-major [64, T4, 32] so the pack-transpose weights AP is
            # contiguous (BIR requires a collapsible stationary AP)
            qt_s = const.tile([128, T4, 32], b16, tag="qt")   # rows 0:64 used
            kt_s = const.tile([128, T4, 32], b16, tag="kt")
            vt_s = const.tile([128, 17, T4], b16, tag="vt")
            # odd-h staging for V (slot-major, rows 64:128 used); Q/K stage
            # their odd half inside qt_s/kt_s rows 64:128 at slot j
            odd_sv = const.tile([128, 8, T4], b16, tag="oddv")
            a_st = const.tile([128, TP, 17], b16, tag="ast")
            zr_s = const.tile([128, TP], f32, tag="zr")
            # h-major, all 4 quarters: col = qq*TP + tp, partition = (par, d)
            a_nm = const.tile([128, 16, 4 * TP], b16, tag="anorm")

            # zero pad slots (s = 16..32) of QT/KT once; ones slot for V once
            nc.any.memset(qt_s[0:64, :, 16:32], 0.0)
            nc.any.memset(kt_s[0:64, :, 16:32], 0.0)
            nc.any.memset(vt_s[0:64, 16, :], 1.0)

            def evac(eng, dst, src, bias):
                if eng == "v":
                    nc.vector.tensor_scalar_add(dst, src, bias)
                else:
                    nc.scalar.activation(
                        dst, src, mybir.ActivationFunctionType.Identity,
                        bias=bias, scale=1.0)

            def oproj_block(qx, m):
                # O-projection m-block over quarter qx (a_nm cols qx*TP..).
                # per h: even-parity matmul on PE rows 0:64, odd on rows
                # 64:128 (wo_s rows 64:128 duplicate 0:64) -> alternating
                # row-groups let each LDWEIGHTS hide under the other matmul's
                # stream.  Own 2-bank PSUM pool so attention batches keep
                # their 6 banks.  One m-block is emitted between consecutive
                # attention batches: the PE chews it while a batch waits for
                # its slot-shift DMA chunk.
                csl = slice(qx * TP, (qx + 1) * TP)
                po_e = pso.tile([128, TP], f32, tag="pso", name="poe")
                po_o = pso.tile([128, TP], f32, tag="pso", name="poo")
                for h in range(16):
                    nc.tensor.matmul(
                        po_e, wo_s[0:64, h, m * 128:(m + 1) * 128],
                        a_nm[0:64, h, csl],
                        start=(h == 0), stop=(h == 15))
                    nc.tensor.matmul(
                        po_o, wo_s[64:128, h, m * 128:(m + 1) * 128],
                        a_nm[64:128, h, csl],
                        start=(h == 0), stop=(h == 15))
                o_e = outp.tile([128, TP], b16, tag="o", name="oe")
                o_o = outp.tile([128, TP], b16, tag="o", name="oo")
                nc.scalar.activation(
                    o_e[:], po_e[:], mybir.ActivationFunctionType.Identity,
                    bias=bo_s[:, m:m + 1], scale=1.0)
                nc.scalar.activation(
                    o_o[:], po_o[:], mybir.ActivationFunctionType.Identity,
                    bias=bo_s[:, m:m + 1], scale=1.0)
                nc.sync.dma_start(out=out_d[m, 0, :, csl], in_=o_e[:])
                nc.sync.dma_start(out=out_d[m, 1, :, csl], in_=o_o[:])

            for qq in range(NQ):
                # ---------- projections ----------
                # evac engines: each tensor's even/odd halves go to different
                # engines so Scalar+Vector chew each j-block concurrently;
                # odd-slot partition-shift DMA issued per-j so it pipelines.
                for x_t, w_s, b_s, dst, tmaj, eng_ev, eng_od in (
                    (xt["xq", qq % 2], wq_s, bq_s, qt_s, True, "v", "s"),
                    (xt["xk", qq % 2], wk_s, bk_s, kt_s, True, "s", "v"),
                    (xt["xv", qq % 2], wv_s, bv_s, vt_s, False, "s", "v"),
                ):
                    for j in range(8):
                        pj = ps.tile([128, 512], f32, tag="ps", name="pj")[:, :T4]
                        for ko in range(8):
                            nc.tensor.matmul(
                                pj, w_s[:, ko, j * 128:(j + 1) * 128],
                                x_t[:, ko, :],
                                start=(ko == 0), stop=(ko == 7))
                        if tmaj:
                            # full-width evacs in two t-halves: rows 0:64
                            # (h=2j) land at slot j directly; rows 64:128
                            # (h=2j+1) stage at slot j of partitions 64:128
                            # (uniform AP); shift DMAs fold them to slots
                            # 8:16 in t-chunks so attention batch 0 only
                            # waits for chunk 0
                            evac(eng_ev, dst[:, 0:128, j], pj[:, 0:128],
                                 b_s[:, j:j + 1])
                            evac(eng_od, dst[:, 128:256, j], pj[:, 128:256],
                                 b_s[:, j:j + 1])
                        else:
                            evac(eng_ev, dst[0:64, j, :], pj[0:64, :],
                                 b_s[0:64, j:j + 1])
                            evac(eng_od, odd_sv[64:128, j, :], pj[64:128, :],
                                 b_s[64:128, j:j + 1])
                    # partition-shifting SBUF->SBUF DMAs for slots 8..16
                    # (emitted right after this tensor's evacs so the drain
                    # overlaps the next tensor's projection)
                    if tmaj:
                        for c in range(4):
                            cs = slice(64 * c, 64 * c + 64)
                            nc.sync.dma_start(
                                out=dst[0:64, cs, 8:16],
                                in_=dst[64:128, cs, 0:8])
                    else:
                        nc.sync.dma_start(
                            out=dst[0:64, 8:16, :], in_=odd_sv[64:128, :, :])
                # duplicate V rows (incl. ones slot) to partitions 64:128
                nc.sync.dma_start(out=vt_s[64:128, :, :], in_=vt_s[0:64, :, :])

                # next quarter's x prefetch + (q0) wo: emitted AFTER the
                # shift DMAs so their bulk descriptors don't delay the
                # shift drain that gates this quarter's attention; still a
                # full quarter ahead of their consumers
                if qq + 1 < NQ:
                    load_x("xq", xq_d, qq + 1)
                    load_x("xk", xk_d, qq + 1)
                    load_x("xv", xv_d, qq + 1)
                if qq == 0:
                    nc.sync.dma_start(out=wo_s[:], in_=wo_d[:])
                    nc.sync.dma_start(
                        out=bo_s[:], in_=bo_d.rearrange("j p -> p j"))



                # ---------- attention ----------
                for b in range(NB):          # 8 batches x 8 groups x 4 tokens
                    qpk_ps = ps.tile([128, 512], b16, tag="ps", name="qpk_ps")
                    kpk_ps = ps.tile([128, 512], b16, tag="ps", name="kpk_ps")
                    for gi in range(8):
                        g = 8 * b + gi
                        for src, pdst in ((qt_s, qpk_ps), (kt_s, kpk_ps)):
                            in_ = src[0:64, 4 * g:4 * g + 4, :]  # [64, 4, 32]
                            nc.tensor.transpose(
                                pdst[:, 64 * gi:64 * gi + 64], in_,
                                ident[0:64, 0:64])
                    qpk = work.tile([128, 512], b16, tag="qpk")
                    kpk = work.tile([128, 512], b16, tag="kpk")
                    nc.vector.tensor_copy(qpk[:], qpk_ps[:])
                    nc.vector.tensor_copy(kpk[:], kpk_ps[:])

                    # each token tau gets a unique (partition-half, PSUM bank):
                    # concurrent matmul drains/clears into the same bank+rows
                    # are a hardware race (observed fatal on device)
                    et_b = [ps.tile([128, 512], f32, tag="ps", name="et0"),
                            ps.tile([128, 512], f32, tag="ps", name="et1")]
                    for gi in range(8):
                        for tau in range(4):
                            nc.tensor.matmul(
                                et_b[tau // 2][64 * (tau % 2):64 * (tau % 2) + 64,
                                               64 * gi:64 * gi + 64],
                                kpk[32 * tau:32 * tau + 32,
                                    64 * gi:64 * gi + 64],
                                qpk[32 * tau:32 * tau + 32,
                                    64 * gi:64 * gi + 64],
                                start=True, stop=True,
                                tile_position=(32 * tau, 64 * (tau % 2)))
                    e_b = [epool.tile([128, 512], b16, tag="e0", name="e0"),
                           epool.tile([128, 512], b16, tag="e1", name="e1")]
                    nc.scalar.activation(e_b[0][:], et_b[0][:],
                                         mybir.ActivationFunctionType.Exp)
                    nc.scalar.activation(e_b[1][:], et_b[1][:],
                                         mybir.ActivationFunctionType.Exp)
                    pa_b = [ps.tile([128, 8, 17], f32, tag="ps", name="pa0"),
                            ps.tile([128, 8, 17], f32, tag="ps", name="pa1")]
                    for gi in range(8):
                        for tau in range(4):
                            t = 32 * b + 4 * gi + tau
                            par = tau % 2
                            nc.tensor.matmul(
                                pa_b[tau // 2][64 * par:64 * par + 64, gi, :],
                                e_b[tau // 2][64 * par:64 * par + 64,
                                              64 * gi:64 * gi + 64],
                                vt_s[64 * par:64 * par + 64, :, t],
                                start=True, stop=True)
                    # tp = 16b + 2gi + tau//2 -> even/odd interleave per bank
                    nc.vector.tensor_copy(
                        a_st[:, 16 * b:16 * b + 16:2, :], pa_b[0][:])
                    nc.vector.tensor_copy(
                        a_st[:, 16 * b + 1:16 * b + 16:2, :], pa_b[1][:])

                    # previous quarter's O-projection m-block: PE filler for
                    # this quarter's next batch's shift-chunk wait
                    if qq > 0:
                        oproj_block(qq - 1, b)

                # ---------- normalize (h-major into the all-quarter a_nm) ----
                nc.vector.reciprocal(zr_s[:], a_st[:, :, 16])
                for h in range(16):
                    nc.gpsimd.tensor_mul(
                        a_nm[:, h, qq * TP:(qq + 1) * TP],
                        a_st[:, :, h], zr_s[:, :])

            # last quarter's O-projection (the only non-overlapped one)
            for m in range(8):
                oproj_block(NQ - 1, m)
    nc.compile()
    return nc


def host_prep(q, k, v, w_q, b_q, w_k, b_k, w_v, b_v, w_o, b_o):
    j = np.arange(8)[:, None, None]
    hb = np.arange(2)[None, :, None]
    d = np.arange(64)[None, None, :]
    perm = (d * 16 + 2 * j + hb).reshape(-1)

    def prep_w(w, scale=1.0):
        wt = (w[perm, :].T.astype(np.float32) * scale).astype(bf16)
        return np.ascontiguousarray(wt.reshape(8, 128, 1024))

    com = dict(
        wq=prep_w(w_q, 0.125), wk=prep_w(w_k), wv=prep_w(w_v),
        bq=np.ascontiguousarray((b_q[perm] * 0.125).reshape(8, 128)).astype(np.float32),
        bk=np.ascontiguousarray(b_k[perm].reshape(8, 128)).astype(np.float32),
        bv=np.ascontiguousarray(b_v[perm].reshape(8, 128)).astype(np.float32),
        bo=np.ascontiguousarray(b_o.reshape(8, 128)).astype(np.float32),
    )
    # V slot order: slot j = h 2j (j<8), slot 8+j = h 2j+1
    hmap = np.array([2 * j for j in range(8)] + [2 * j + 1 for j in range(8)])
    wo_half = np.transpose(w_o.reshape(1024, 64, 16), (1, 2, 0))[:, hmap, :]
    com["wo"] = np.ascontiguousarray(
        np.concatenate([wo_half, wo_half], axis=0).astype(bf16))

    in_maps = []
    for c in range(NCORE):
        m = dict(com)
        for name, x in (("xq", q), ("xk", k), ("xv", v)):
            sl = x.reshape(-1, D)[c * T:(c + 1) * T, :]
            m[name] = np.ascontiguousarray(sl.T.astype(bf16).reshape(8, 128, T))
        in_maps.append(m)
    return in_maps


def reassemble(results):
    # per-core out [8, 2, 128, 512] -> [B, S, D]
    full = np.empty((NCORE, T, D), np.float32)
    for c, res in enumerate(results):
        od = res["out"]                     # [m=8, par=2, p=128, col=512]
        # col = qq*TP + tp ; token t = qq*T4 + 2*tp + par ; D = m*128 + p
        o = np.transpose(od, (3, 1, 0, 2))  # [col, par, m, p]
        full[c] = o.reshape(4, TP, 2, D).reshape(4, T4, D).reshape(T, D)
    return full.reshape(B, S, D)


def kernel(**inputs):
    from concourse.bass_utils import run_bass_kernel_spmd
    if "nc" not in _NC_CACHE:
        _NC_CACHE["nc"] = build_nc()
    nc = _NC_CACHE["nc"]
    in_maps = host_prep(**inputs)
    r = run_bass_kernel_spmd(nc, in_maps, core_ids=list(range(NCORE)))
    return reassemble(r.results)


if __name__ == "__main__":
    z = np.load("/root/problem/inputs_cache.npz")
    inputs = {kk: z[kk] for kk in z.files}
    expd = np.load("/root/problem/expected64.npy")
    act = kernel(**inputs)
    err = np.abs(act - expd)
    scale = np.abs(expd).max()
    print("absmax err:", err.max(), "rel:", err.max() / scale)



# revision 60
# speedup vs baseline: 1.1832x; 1.0228x over previous
"""Trainium2 Bass kernel for nn_MultiHeadAttention_59158879535767.

Reference semantics (B=4, S=2048, D=1024, H=16, DK=64):
  Q = q @ w_q.T + b_q  (same for K, V), reshaped (B,S,DK,H);
  score contracts over the HEAD axis per token: score[t] = Q_t @ K_t.T / 8
  (64x64 per token), softmax over last axis, attn[t] = score @ V_t -> (64,16),
  flattened, then @ w_o.T + b_o.

Everything is per-token => data-parallel over the 8192 tokens across 8 cores,
no collectives. Per core T=1024 tokens, processed in NQ=4 quarters of T4=256.

On-device dataflow per core (all matmuls bf16 with fp32 PSUM accumulation):
  * QKV projections: lhsT = host-permuted W.T tiles [din, (j, hb, d)] where
    output column j*128 + hb*64 + d holds dout = d*16 + (2j+hb). rhs = host-
    transposed x.T [din, tok]. PSUM [128=(hb,d), T4]; bias folded at evac.
    Q/K evac: ONE full-width [128, 128] op per (j, t-half) (rows 0:64 land
    at slot j; rows 64:128 stage at slot j of partitions 64:128 of the same
    tile - uniform AP); per-32-token-chunk SBUF->SBUF DMAs then shift the
    staged odd half down to slots 8:16 (engines cannot move partitions).
    Evac halves alternate Scalar/Vector; weight loads are staggered
    (first j-blocks land early) and x is prefetched one quarter ahead so
    its descriptors beat the shift-DMA flood into the queues.
  * Layouts: QT/KT [128, T4, 32 s] (s = h slot; rows 0:64 data, 16..31
    zeroed; rows 64:128 odd-h staging), VT [128, 17, T4] (slot 16 = ones,
    rows 64:128 duplicate of 0:64 via DMA).
  * S1 per 4-token group: one PE transpose each of QT/KT [64, (tau,s)=128]
    -> pack [128=(tau,s), 64] in PSUM (8 groups batched per bank), evac to
    SBUF. Then per token tau: matmul K=32 rows at base 32*tau:
      lhsT=Kpk[32t:+32, 64g:+64], rhs=Qpk[...] -> ET [64 e, 64 d] at
      (64*(tau%2) partitions, 64*(tau//2) free) of a [128,128] PSUM quarter.
  * exp via ACT on [128, 512] (4 groups) -> E bf16. No max subtraction needed
    (|score| <= ~3 for this distribution).
  * S2 per token: lhsT = E-slice [64 e, 64 d], rhs = VT_eh1d[64q:+64, :, t]
    [64, 17] -> out [64 d, 17] (slot 16 = sum of exp = softmax denominator).
  * normalize: A_norm = A[:, :, 0:16] * recip(A[:, :, 16]) -> bf16, written
    h-major into the all-quarter a_nm [128, 16 h, 512 t] (muls on GpSimd).
  * O-projection: deferred per-(quarter, m) blocks; per h one even-parity
    matmul on PE rows 0:64 and one odd-parity on rows 64:128 (wo rows
    64:128 duplicate 0:64) so every LDWEIGHTS hides under the other
    parity's stream; blocks of quarter q-1 are interleaved between quarter
    q's attention batches as PE stall-filler; + b_o at evac -> out DRAM
    [8 m, 2 par, 128, 512] bf16, host reassembles.
"""
import numpy as np
import ml_dtypes

B, S, D, H, DK = 4, 2048, 1024, 16, 64
NCORE = 8
T = (B * S) // NCORE          # 1024 tokens per core
NQ = 4
T4 = T // NQ                  # 256 tokens per quarter
TP = T4 // 2                  # 128 tokens per parity per quarter
NB = T4 // 32                 # 8 batches of 8 groups (32 tokens) per quarter

bf16 = ml_dtypes.bfloat16

_NC_CACHE = {}


def build_nc():
    import concourse.bacc as bacc
    import concourse.mybir as mybir
    import concourse.tile as tile
    from concourse.masks import make_identity

    nc = bacc.Bacc()
    dt = mybir.dt
    f32, b16 = dt.float32, dt.bfloat16

    # ---- DRAM I/O ----
    xq_d = nc.dram_tensor("xq", [8, 128, T], b16, kind="ExternalInput")
    xk_d = nc.dram_tensor("xk", [8, 128, T], b16, kind="ExternalInput")
    xv_d = nc.dram_tensor("xv", [8, 128, T], b16, kind="ExternalInput")
    wq_d = nc.dram_tensor("wq", [8, 128, 1024], b16, kind="ExternalInput")
    wk_d = nc.dram_tensor("wk", [8, 128, 1024], b16, kind="ExternalInput")
    wv_d = nc.dram_tensor("wv", [8, 128, 1024], b16, kind="ExternalInput")
    wo_d = nc.dram_tensor("wo", [128, 16, 1024], b16, kind="ExternalInput")
    bq_d = nc.dram_tensor("bq", [8, 128], f32, kind="ExternalInput")
    bk_d = nc.dram_tensor("bk", [8, 128], f32, kind="ExternalInput")
    bv_d = nc.dram_tensor("bv", [8, 128], f32, kind="ExternalInput")
    bo_d = nc.dram_tensor("bo", [8, 128], f32, kind="ExternalInput")
    out_d = nc.dram_tensor("out", [8, 2, 128, 4 * TP], b16, kind="ExternalOutput")

    with tile.TileContext(nc) as tc:
        with (
            tc.tile_pool(name="const", bufs=1) as const,
            tc.tile_pool(name="xin", bufs=2) as xin,
            tc.tile_pool(name="work", bufs=2) as work,
            tc.tile_pool(name="epool", bufs=3) as epool,
            tc.tile_pool(name="outp", bufs=2) as outp,
            tc.tile_pool(name="ps", bufs=6, space="PSUM") as ps,
            tc.tile_pool(name="pso", bufs=2, space="PSUM") as pso,
        ):
            # ---- persistent SBUF ----
            wq_s = const.tile([128, 8, 1024], b16, tag="wq")
            wk_s = const.tile([128, 8, 1024], b16, tag="wk")
            wv_s = const.tile([128, 8, 1024], b16, tag="wv")
            wo_s = const.tile([128, 16, 1024], b16, tag="wo")
            bq_s = const.tile([128, 8], f32, tag="bq")
            bk_s = const.tile([128, 8], f32, tag="bk")
            bv_s = const.tile([128, 8], f32, tag="bv")
            bo_s = const.tile([128, 8], f32, tag="bo")
            ident = const.tile([128, 128], b16, tag="ident")
            make_identity(nc, ident)

            # x double-buffers (explicit, so quarter-0 loads can interleave
            # with the weight loads: Q-proj work starts as early as possible)
            xt = {}
            for nm in ("xq", "xk", "xv"):
                for pb in range(2):
                    xt[nm, pb] = xin.tile([128, 8, T4], b16,
                                          tag=f"{nm}{pb}", name=f"{nm}{pb}")

            def load_x(nm, xd, qq):
                tsl = slice(qq * T4, (qq + 1) * T4)
                nc.sync.dma_start(
                    out=xt[nm, qq % 2][:],
                    in_=xd[:, :, tsl].rearrange("ko p t -> p ko t"))

            # staggered weight loads: each tensor's first j-blocks (m 0:256)
            # land early so its projection starts while the rest streams
            load_x("xq", xq_d, 0)
            wq_r = wq_d.rearrange("ko p m -> p ko m")
            wk_r = wk_d.rearrange("ko p m -> p ko m")
            wv_r = wv_d.rearrange("ko p m -> p ko m")
            nc.sync.dma_start(out=wq_s[:, :, 0:256], in_=wq_r[:, :, 0:256])
            nc.sync.dma_start(out=bq_s[:], in_=bq_d.rearrange("j p -> p j"))
            nc.sync.dma_start(out=wq_s[:, :, 256:1024], in_=wq_r[:, :, 256:1024])
            load_x("xk", xk_d, 0)
            nc.sync.dma_start(out=wk_s[:, :, 0:256], in_=wk_r[:, :, 0:256])
            nc.sync.dma_start(out=bk_s[:], in_=bk_d.rearrange("j p -> p j"))
            nc.sync.dma_start(out=wk_s[:, :, 256:1024], in_=wk_r[:, :, 256:1024])
            load_x("xv", xv_d, 0)
            nc.sync.dma_start(out=wv_s[:, :, 0:256], in_=wv_r[:, :, 0:256])
            nc.sync.dma_start(out=bv_s[:], in_=bv_d.rearrange("j p -> p j"))
            nc.sync.dma_start(out=wv_s[:, :, 256:1024], in_=wv_r[:, :, 256:1024])
            # wo/bo are first needed by oproj (mid-quarter-2); loaded inside
            # quarter 0's body so the startup burst isn't bandwidth-starved

            # token-major [64, T4, 32] so the pack-transpose weights AP is
            # contiguous (BIR requires a collapsible stationary AP)
            qt_s = const.tile([128, T4, 32], b16, tag="qt")   # rows 0:64 used
            kt_s = const.tile([128, T4, 32], b16, tag="kt")
            vt_s = const.tile([128, 17, T4], b16, tag="vt")
            # odd-h staging for V (slot-major, rows 64:128 used); Q/K stage
            # their odd half inside qt_s/kt_s rows 64:128 at slot j
            odd_sv = const.tile([128, 8, T4], b16, tag="oddv")
            a_st = const.tile([128, TP, 17], b16, tag="ast")
            zr_s = const.tile([128, TP], f32, tag="zr")
            # h-major, all 4 quarters: col = qq*TP + tp, partition = (par, d)
            a_nm = const.tile([128, 16, 4 * TP], b16, tag="anorm")

            # zero pad slots (s = 16..32) of QT/KT once; ones slot for V once
            nc.any.memset(qt_s[0:64, :, 16:32], 0.0)
            nc.any.memset(kt_s[0:64, :, 16:32], 0.0)
            nc.any.memset(vt_s[0:64, 16, :], 1.0)

            def evac(eng, dst, src, bias):
                if eng == "v":
                    nc.vector.tensor_scalar_add(dst, src, bias)
                else:
                    nc.scalar.activation(
                        dst, src, mybir.ActivationFunctionType.Identity,
                        bias=bias, scale=1.0)

            def oproj_block(qx, m):
                # O-projection m-block over quarter qx (a_nm cols qx*TP..).
                # per h: even-parity matmul on PE rows 0:64, odd on rows
                # 64:128 (wo_s rows 64:128 duplicate 0:64) -> alternating
                # row-groups let each LDWEIGHTS hide under the other matmul's
                # stream.  Own 2-bank PSUM pool so attention batches keep
                # their 6 banks.  One m-block is emitted between consecutive
                # attention batches: the PE chews it while a batch waits for
                # its slot-shift DMA chunk.
                csl = slice(qx * TP, (qx + 1) * TP)
                po_e = pso.tile([128, TP], f32, tag="pso", name="poe")
                po_o = pso.tile([128, TP], f32, tag="pso", name="poo")
                for h in range(16):
                    nc.tensor.matmul(
                        po_e, wo_s[0:64, h, m * 128:(m + 1) * 128],
                        a_nm[0:64, h, csl],
                        start=(h == 0), stop=(h == 15))
                    nc.tensor.matmul(
                        po_o, wo_s[64:128, h, m * 128:(m + 1) * 128],
                        a_nm[64:128, h, csl],
                        start=(h == 0), stop=(h == 15))
                o_e = outp.tile([128, TP], b16, tag="o", name="oe")
                o_o = outp.tile([128, TP], b16, tag="o", name="oo")
                nc.scalar.activation(
                    o_e[:], po_e[:], mybir.ActivationFunctionType.Identity,
                    bias=bo_s[:, m:m + 1], scale=1.0)
                nc.scalar.activation(
                    o_o[:], po_o[:], mybir.ActivationFunctionType.Identity,
                    bias=bo_s[:, m:m + 1], scale=1.0)
                nc.sync.dma_start(out=out_d[m, 0, :, csl], in_=o_e[:])
                nc.sync.dma_start(out=out_d[m, 1, :, csl], in_=o_o[:])

            for qq in range(NQ):
                # ---------- projections ----------
                # evac engines: each tensor's even/odd halves go to different
                # engines so Scalar+Vector chew each j-block concurrently;
                # odd-slot partition-shift DMA issued per-j so it pipelines.
                for x_t, w_s, b_s, dst, tmaj, eng_ev, eng_od in (
                    (xt["xq", qq % 2], wq_s, bq_s, qt_s, True, "v", "s"),
                    (xt["xk", qq % 2], wk_s, bk_s, kt_s, True, "s", "v"),
                    (xt["xv", qq % 2], wv_s, bv_s, vt_s, False, "s", "v"),
                ):
                    for j in range(8):
                        pj = ps.tile([128, 512], f32, tag="ps", name="pj")[:, :T4]
                        for ko in range(8):
                            nc.tensor.matmul(
                                pj, w_s[:, ko, j * 128:(j + 1) * 128],
                                x_t[:, ko, :],
                                start=(ko == 0), stop=(ko == 7))
                        if tmaj:
                            # full-width evacs in two t-halves: rows 0:64
                            # (h=2j) land at slot j directly; rows 64:128
                            # (h=2j+1) stage at slot j of partitions 64:128
                            # (uniform AP); shift DMAs fold them to slots
                            # 8:16 in t-chunks so attention batch 0 only
                            # waits for chunk 0
                            evac(eng_ev, dst[:, 0:128, j], pj[:, 0:128],
                                 b_s[:, j:j + 1])
                            evac(eng_od, dst[:, 128:256, j], pj[:, 128:256],
                                 b_s[:, j:j + 1])
                        else:
                            evac(eng_ev, dst[0:64, j, :], pj[0:64, :],
                                 b_s[0:64, j:j + 1])
                            evac(eng_od, odd_sv[64:128, j, :], pj[64:128, :],
                                 b_s[64:128, j:j + 1])
                    # partition-shifting SBUF->SBUF DMAs for slots 8..16
                    # (emitted right after this tensor's evacs so the drain
                    # overlaps the next tensor's projection)
                    if tmaj:
                        for c in range(4):
                            cs = slice(64 * c, 64 * c + 64)
                            nc.sync.dma_start(
                                out=dst[0:64, cs, 8:16],
                                in_=dst[64:128, cs, 0:8])
                    else:
                        nc.sync.dma_start(
                            out=dst[0:64, 8:16, :], in_=odd_sv[64:128, :, :])
                # duplicate V rows (incl. ones slot) to partitions 64:128
                nc.sync.dma_start(out=vt_s[64:128, :, :], in_=vt_s[0:64, :, :])

                # next quarter's x prefetch + (q0) wo: emitted AFTER the
                # shift DMAs so their bulk descriptors don't delay the
                # shift drain that gates this quarter's attention; still a
                # full quarter ahead of their consumers
                if qq + 1 < NQ:
                    load_x("xq", xq_d, qq + 1)
                    load_x("xk", xk_d, qq + 1)
                    load_x("xv", xv_d, qq + 1)
                if qq == 0:
                    nc.sync.dma_start(out=wo_s[:], in_=wo_d[:])
                    nc.sync.dma_start(
                        out=bo_s[:], in_=bo_d.rearrange("j p -> p j"))



                # ---------- attention ----------
                for b in range(NB):          # 8 batches x 8 groups x 4 tokens
                    qpk_ps = ps.tile([128, 512], b16, tag="ps", name="qpk_ps")
                    kpk_ps = ps.tile([128, 512], b16, tag="ps", name="kpk_ps")
                    # all Q transposes BEFORE the K ones: K's shift chunks
                    # land later than Q's, so the Q batch executes while the
                    # K shift DMA is still draining
                    for src, pdst in ((qt_s, qpk_ps), (kt_s, kpk_ps)):
                        for gi in range(8):
                            g = 8 * b + gi
                            in_ = src[0:64, 4 * g:4 * g + 4, :]  # [64, 4, 32]
                            nc.tensor.transpose(
                                pdst[:, 64 * gi:64 * gi + 64], in_,
                                ident[0:64, 0:64])
                    qpk = work.tile([128, 512], b16, tag="qpk")
                    kpk = work.tile([128, 512], b16, tag="kpk")
                    nc.vector.tensor_copy(qpk[:], qpk_ps[:])
                    nc.vector.tensor_copy(kpk[:], kpk_ps[:])

                    # each token tau gets a unique (partition-half, PSUM bank):
                    # concurrent matmul drains/clears into the same bank+rows
                    # are a hardware race (observed fatal on device)
                    et_b = [ps.tile([128, 512], f32, tag="ps", name="et0"),
                            ps.tile([128, 512], f32, tag="ps", name="et1")]
                    for gi in range(8):
                        for tau in range(4):
                            nc.tensor.matmul(
                                et_b[tau // 2][64 * (tau % 2):64 * (tau % 2) + 64,
                                               64 * gi:64 * gi + 64],
                                kpk[32 * tau:32 * tau + 32,
                                    64 * gi:64 * gi + 64],
                                qpk[32 * tau:32 * tau + 32,
                                    64 * gi:64 * gi + 64],
                                start=True, stop=True,
                                tile_position=(32 * tau, 64 * (tau % 2)))
                    e_b = [epool.tile([128, 512], b16, tag="e0", name="e0"),
                           epool.tile([128, 512], b16, tag="e1", name="e1")]
                    nc.scalar.activation(e_b[0][:], et_b[0][:],
                                         mybir.ActivationFunctionType.Exp)
                    nc.scalar.activation(e_b[1][:], et_b[1][:],
                                         mybir.ActivationFunctionType.Exp)
                    pa_b = [ps.tile([128, 8, 17], f32, tag="ps", name="pa0"),
                            ps.tile([128, 8, 17], f32, tag="ps", name="pa1")]
                    for gi in range(8):
                        for tau in range(4):
                            t = 32 * b + 4 * gi + tau
                            par = tau % 2
                            nc.tensor.matmul(
                                pa_b[tau // 2][64 * par:64 * par + 64, gi, :],
                                e_b[tau // 2][64 * par:64 * par + 64,
                                              64 * gi:64 * gi + 64],
                                vt_s[64 * par:64 * par + 64, :, t],
                                start=True, stop=True)
                    # tp = 16b + 2gi + tau//2 -> even/odd interleave per bank
                    nc.vector.tensor_copy(
                        a_st[:, 16 * b:16 * b + 16:2, :], pa_b[0][:])
                    nc.vector.tensor_copy(
                        a_st[:, 16 * b + 1:16 * b + 16:2, :], pa_b[1][:])

                    # previous quarter's O-projection m-block: PE filler for
                    # this quarter's next batch's shift-chunk wait
                    if qq > 0:
                        oproj_block(qq - 1, b)

                # ---------- normalize (h-major into the all-quarter a_nm) ----
                nc.vector.reciprocal(zr_s[:], a_st[:, :, 16])
                for h in range(16):
                    nc.gpsimd.tensor_mul(
                        a_nm[:, h, qq * TP:(qq + 1) * TP],
                        a_st[:, :, h], zr_s[:, :])

            # last quarter's O-projection (the only non-overlapped one)
            for m in range(8):
                oproj_block(NQ - 1, m)
    nc.compile()
    return nc


def host_prep(q, k, v, w_q, b_q, w_k, b_k, w_v, b_v, w_o, b_o):
    j = np.arange(8)[:, None, None]
    hb = np.arange(2)[None, :, None]
    d = np.arange(64)[None, None, :]
    perm = (d * 16 + 2 * j + hb).reshape(-1)

    def prep_w(w, scale=1.0):
        wt = (w[perm, :].T.astype(np.float32) * scale).astype(bf16)
        return np.ascontiguousarray(wt.reshape(8, 128, 1024))

    com = dict(
        wq=prep_w(w_q, 0.125), wk=prep_w(w_k), wv=prep_w(w_v),
        bq=np.ascontiguousarray((b_q[perm] * 0.125).reshape(8, 128)).astype(np.float32),
        bk=np.ascontiguousarray(b_k[perm].reshape(8, 128)).astype(np.float32),
        bv=np.ascontiguousarray(b_v[perm].reshape(8, 128)).astype(np.float32),
        bo=np.ascontiguousarray(b_o.reshape(8, 128)).astype(np.float32),
    )
    # V slot order: slot j = h 2j (j<8), slot 8+j = h 2j+1
    hmap = np.array([2 * j for j in range(8)] + [2 * j + 1 for j in range(8)])
    wo_half = np.transpose(w_o.reshape(1024, 64, 16), (1, 2, 0))[:, hmap, :]
    com["wo"] = np.ascontiguousarray(
        np.concatenate([wo_half, wo_half], axis=0).astype(bf16))

    in_maps = []
    for c in range(NCORE):
        m = dict(com)
        for name, x in (("xq", q), ("xk", k), ("xv", v)):
            sl = x.reshape(-1, D)[c * T:(c + 1) * T, :]
            m[name] = np.ascontiguousarray(sl.T.astype(bf16).reshape(8, 128, T))
        in_maps.append(m)
    return in_maps


def reassemble(results):
    # per-core out [8, 2, 128, 512] -> [B, S, D]
    full = np.empty((NCORE, T, D), np.float32)
    for c, res in enumerate(results):
        od = res["out"]                     # [m=8, par=2, p=128, col=512]
        # col = qq*TP + tp ; token t = qq*T4 + 2*tp + par ; D = m*128 + p
        o = np.transpose(od, (3, 1, 0, 2))  # [col, par, m, p]
        full[c] = o.reshape(4, TP, 2, D).reshape(4, T4, D).reshape(T, D)
    return full.reshape(B, S, D)


def kernel(**inputs):
    from concourse.bass_utils import run_bass_kernel_spmd
    if "nc" not in _NC_CACHE:
        _NC_CACHE["nc"] = build_nc()
    nc = _NC_CACHE["nc"]
    in_maps = host_prep(**inputs)
    r = run_bass_kernel_spmd(nc, in_maps, core_ids=list(range(NCORE)))
    return reassemble(r.results)


if __name__ == "__main__":
    z = np.load("/root/problem/inputs_cache.npz")
    inputs = {kk: z[kk] for kk in z.files}
    expd = np.load("/root/problem/expected64.npy")
    act = kernel(**inputs)
    err = np.abs(act - expd)
    scale = np.abs(expd).max()
    print("absmax err:", err.max(), "rel:", err.max() / scale)



# revision 61
# speedup vs baseline: 1.1865x; 1.0028x over previous
"""Trainium2 Bass kernel for nn_MultiHeadAttention_59158879535767.

Reference semantics (B=4, S=2048, D=1024, H=16, DK=64):
  Q = q @ w_q.T + b_q  (same for K, V), reshaped (B,S,DK,H);
  score contracts over the HEAD axis per token: score[t] = Q_t @ K_t.T / 8
  (64x64 per token), softmax over last axis, attn[t] = score @ V_t -> (64,16),
  flattened, then @ w_o.T + b_o.

Everything is per-token => data-parallel over the 8192 tokens across 8 cores,
no collectives. Per core T=1024 tokens, processed in NQ=4 quarters of T4=256.

On-device dataflow per core (all matmuls bf16 with fp32 PSUM accumulation):
  * QKV projections: lhsT = host-permuted W.T tiles [din, (j, hb, d)] where
    output column j*128 + hb*64 + d holds dout = d*16 + (2j+hb). rhs = host-
    transposed x.T [din, tok]. PSUM [128=(hb,d), T4]; bias folded at evac.
    Q/K evac: ONE full-width [128, 128] op per (j, t-half) (rows 0:64 land
    at slot j; rows 64:128 stage at slot j of partitions 64:128 of the same
    tile - uniform AP); per-32-token-chunk SBUF->SBUF DMAs then shift the
    staged odd half down to slots 8:16 (engines cannot move partitions).
    Evac halves alternate Scalar/Vector; weight loads are staggered
    (first j-blocks land early) and x is prefetched one quarter ahead so
    its descriptors beat the shift-DMA flood into the queues.
  * Layouts: QT/KT [128, T4, 32 s] (s = h slot; rows 0:64 data, 16..31
    zeroed; rows 64:128 odd-h staging), VT [128, 17, T4] (slot 16 = ones,
    rows 64:128 duplicate of 0:64 via DMA).
  * S1 per 4-token group: one PE transpose each of QT/KT [64, (tau,s)=128]
    -> pack [128=(tau,s), 64] in PSUM (8 groups batched per bank), evac to
    SBUF. Then per token tau: matmul K=32 rows at base 32*tau:
      lhsT=Kpk[32t:+32, 64g:+64], rhs=Qpk[...] -> ET [64 e, 64 d] at
      (64*(tau%2) partitions, 64*(tau//2) free) of a [128,128] PSUM quarter.
  * exp via ACT on [128, 512] (4 groups) -> E bf16. No max subtraction needed
    (|score| <= ~3 for this distribution).
  * S2 per token: lhsT = E-slice [64 e, 64 d], rhs = VT_eh1d[64q:+64, :, t]
    [64, 17] -> out [64 d, 17] (slot 16 = sum of exp = softmax denominator).
  * normalize: A_norm = A[:, :, 0:16] * recip(A[:, :, 16]) -> bf16, written
    h-major into the all-quarter a_nm [128, 16 h, 512 t] (muls on GpSimd).
  * O-projection: deferred per-(quarter, m) blocks; per h one even-parity
    matmul on PE rows 0:64 and one odd-parity on rows 64:128 (wo rows
    64:128 duplicate 0:64) so every LDWEIGHTS hides under the other
    parity's stream; blocks of quarter q-1 are interleaved between quarter
    q's attention batches as PE stall-filler; + b_o at evac -> out DRAM
    [8 m, 2 par, 128, 512] bf16, host reassembles.
"""
import numpy as np
import ml_dtypes

B, S, D, H, DK = 4, 2048, 1024, 16, 64
NCORE = 8
T = (B * S) // NCORE          # 1024 tokens per core
NQ = 4
T4 = T // NQ                  # 256 tokens per quarter
TP = T4 // 2                  # 128 tokens per parity per quarter
NB = T4 // 32                 # 8 batches of 8 groups (32 tokens) per quarter

bf16 = ml_dtypes.bfloat16

_NC_CACHE = {}


def build_nc():
    import concourse.bacc as bacc
    import concourse.mybir as mybir
    import concourse.tile as tile
    from concourse.masks import make_identity

    nc = bacc.Bacc()
    dt = mybir.dt
    f32, b16 = dt.float32, dt.bfloat16

    # ---- DRAM I/O ----
    xq_d = nc.dram_tensor("xq", [8, 128, T], b16, kind="ExternalInput")
    xk_d = nc.dram_tensor("xk", [8, 128, T], b16, kind="ExternalInput")
    xv_d = nc.dram_tensor("xv", [8, 128, T], b16, kind="ExternalInput")
    wq_d = nc.dram_tensor("wq", [8, 128, 1024], b16, kind="ExternalInput")
    wk_d = nc.dram_tensor("wk", [8, 128, 1024], b16, kind="ExternalInput")
    wv_d = nc.dram_tensor("wv", [8, 128, 1024], b16, kind="ExternalInput")
    wo_d = nc.dram_tensor("wo", [128, 16, 1024], b16, kind="ExternalInput")
    bq_d = nc.dram_tensor("bq", [8, 128], f32, kind="ExternalInput")
    bk_d = nc.dram_tensor("bk", [8, 128], f32, kind="ExternalInput")
    bv_d = nc.dram_tensor("bv", [8, 128], f32, kind="ExternalInput")
    bo_d = nc.dram_tensor("bo", [8, 128], f32, kind="ExternalInput")
    out_d = nc.dram_tensor("out", [8, 2, 128, 4 * TP], b16, kind="ExternalOutput")

    with tile.TileContext(nc) as tc:
        with (
            tc.tile_pool(name="const", bufs=1) as const,
            tc.tile_pool(name="xin", bufs=2) as xin,
            tc.tile_pool(name="work", bufs=3) as work,
            tc.tile_pool(name="epool", bufs=3) as epool,
            tc.tile_pool(name="outp", bufs=3) as outp,
            tc.tile_pool(name="ps", bufs=6, space="PSUM") as ps,
            tc.tile_pool(name="pso", bufs=2, space="PSUM") as pso,
        ):
            # ---- persistent SBUF ----
            wq_s = const.tile([128, 8, 1024], b16, tag="wq")
            wk_s = const.tile([128, 8, 1024], b16, tag="wk")
            wv_s = const.tile([128, 8, 1024], b16, tag="wv")
            wo_s = const.tile([128, 16, 1024], b16, tag="wo")
            bq_s = const.tile([128, 8], f32, tag="bq")
            bk_s = const.tile([128, 8], f32, tag="bk")
            bv_s = const.tile([128, 8], f32, tag="bv")
            bo_s = const.tile([128, 8], f32, tag="bo")
            ident = const.tile([128, 128], b16, tag="ident")
            make_identity(nc, ident)

            # x double-buffers (explicit, so quarter-0 loads can interleave
            # with the weight loads: Q-proj work starts as early as possible)
            xt = {}
            for nm in ("xq", "xk", "xv"):
                for pb in range(2):
                    xt[nm, pb] = xin.tile([128, 8, T4], b16,
                                          tag=f"{nm}{pb}", name=f"{nm}{pb}")

            def load_x(nm, xd, qq):
                tsl = slice(qq * T4, (qq + 1) * T4)
                nc.sync.dma_start(
                    out=xt[nm, qq % 2][:],
                    in_=xd[:, :, tsl].rearrange("ko p t -> p ko t"))

            # staggered weight loads: each tensor's first j-blocks (m 0:256)
            # land early so its projection starts while the rest streams
            load_x("xq", xq_d, 0)
            wq_r = wq_d.rearrange("ko p m -> p ko m")
            wk_r = wk_d.rearrange("ko p m -> p ko m")
            wv_r = wv_d.rearrange("ko p m -> p ko m")
            nc.sync.dma_start(out=wq_s[:, :, 0:256], in_=wq_r[:, :, 0:256])
            nc.sync.dma_start(out=bq_s[:], in_=bq_d.rearrange("j p -> p j"))
            nc.sync.dma_start(out=wq_s[:, :, 256:1024], in_=wq_r[:, :, 256:1024])
            load_x("xk", xk_d, 0)
            nc.sync.dma_start(out=wk_s[:, :, 0:256], in_=wk_r[:, :, 0:256])
            nc.sync.dma_start(out=bk_s[:], in_=bk_d.rearrange("j p -> p j"))
            nc.sync.dma_start(out=wk_s[:, :, 256:1024], in_=wk_r[:, :, 256:1024])
            load_x("xv", xv_d, 0)
            nc.sync.dma_start(out=wv_s[:, :, 0:256], in_=wv_r[:, :, 0:256])
            nc.sync.dma_start(out=bv_s[:], in_=bv_d.rearrange("j p -> p j"))
            nc.sync.dma_start(out=wv_s[:, :, 256:1024], in_=wv_r[:, :, 256:1024])
            # wo/bo are first needed by oproj (mid-quarter-2); loaded inside
            # quarter 0's body so the startup burst isn't bandwidth-starved

            # token-major [64, T4, 32] so the pack-transpose weights AP is
            # contiguous (BIR requires a collapsible stationary AP)
            qt_s = const.tile([128, T4, 32], b16, tag="qt")   # rows 0:64 used
            kt_s = const.tile([128, T4, 32], b16, tag="kt")
            vt_s = const.tile([128, 17, T4], b16, tag="vt")
            # odd-h staging for V (slot-major, rows 64:128 used); Q/K stage
            # their odd half inside qt_s/kt_s rows 64:128 at slot j
            odd_sv = const.tile([128, 8, T4], b16, tag="oddv")
            a_st = const.tile([128, TP, 17], b16, tag="ast")
            zr_s = const.tile([128, TP], f32, tag="zr")
            # h-major, all 4 quarters: col = qq*TP + tp, partition = (par, d)
            a_nm = const.tile([128, 16, 4 * TP], b16, tag="anorm")

            # zero pad slots (s = 16..32) of QT/KT once; ones slot for V once
            nc.any.memset(qt_s[0:64, :, 16:32], 0.0)
            nc.any.memset(kt_s[0:64, :, 16:32], 0.0)
            nc.any.memset(vt_s[0:64, 16, :], 1.0)

            def evac(eng, dst, src, bias):
                if eng == "v":
                    nc.vector.tensor_scalar_add(dst, src, bias)
                else:
                    nc.scalar.activation(
                        dst, src, mybir.ActivationFunctionType.Identity,
                        bias=bias, scale=1.0)

            def oproj_block(qx, m):
                # O-projection m-block over quarter qx (a_nm cols qx*TP..).
                # per h: even-parity matmul on PE rows 0:64, odd on rows
                # 64:128 (wo_s rows 64:128 duplicate 0:64) -> alternating
                # row-groups let each LDWEIGHTS hide under the other matmul's
                # stream.  Own 2-bank PSUM pool so attention batches keep
                # their 6 banks.  One m-block is emitted between consecutive
                # attention batches: the PE chews it while a batch waits for
                # its slot-shift DMA chunk.
                csl = slice(qx * TP, (qx + 1) * TP)
                po_e = pso.tile([128, TP], f32, tag="pso", name="poe")
                po_o = pso.tile([128, TP], f32, tag="pso", name="poo")
                for h in range(16):
                    nc.tensor.matmul(
                        po_e, wo_s[0:64, h, m * 128:(m + 1) * 128],
                        a_nm[0:64, h, csl],
                        start=(h == 0), stop=(h == 15))
                    nc.tensor.matmul(
                        po_o, wo_s[64:128, h, m * 128:(m + 1) * 128],
                        a_nm[64:128, h, csl],
                        start=(h == 0), stop=(h == 15))
                o_e = outp.tile([128, TP], b16, tag="o", name="oe")
                o_o = outp.tile([128, TP], b16, tag="o", name="oo")
                nc.scalar.activation(
                    o_e[:], po_e[:], mybir.ActivationFunctionType.Identity,
                    bias=bo_s[:, m:m + 1], scale=1.0)
                nc.scalar.activation(
                    o_o[:], po_o[:], mybir.ActivationFunctionType.Identity,
                    bias=bo_s[:, m:m + 1], scale=1.0)
                nc.sync.dma_start(out=out_d[m, 0, :, csl], in_=o_e[:])
                nc.sync.dma_start(out=out_d[m, 1, :, csl], in_=o_o[:])

            for qq in range(NQ):
                # ---------- projections ----------
                # evac engines: each tensor's even/odd halves go to different
                # engines so Scalar+Vector chew each j-block concurrently;
                # odd-slot partition-shift DMA issued per-j so it pipelines.
                for x_t, w_s, b_s, dst, tmaj, eng_ev, eng_od in (
                    (xt["xq", qq % 2], wq_s, bq_s, qt_s, True, "v", "s"),
                    (xt["xk", qq % 2], wk_s, bk_s, kt_s, True, "s", "v"),
                    (xt["xv", qq % 2], wv_s, bv_s, vt_s, False, "s", "v"),
                ):
                    for j in range(8):
                        pj = ps.tile([128, 512], f32, tag="ps", name="pj")[:, :T4]
                        for ko in range(8):
                            nc.tensor.matmul(
                                pj, w_s[:, ko, j * 128:(j + 1) * 128],
                                x_t[:, ko, :],
                                start=(ko == 0), stop=(ko == 7))
                        if tmaj:
                            # full-width evacs in two t-halves: rows 0:64
                            # (h=2j) land at slot j directly; rows 64:128
                            # (h=2j+1) stage at slot j of partitions 64:128
                            # (uniform AP); shift DMAs fold them to slots
                            # 8:16 in t-chunks so attention batch 0 only
                            # waits for chunk 0
                            evac(eng_ev, dst[:, 0:128, j], pj[:, 0:128],
                                 b_s[:, j:j + 1])
                            evac(eng_od, dst[:, 128:256, j], pj[:, 128:256],
                                 b_s[:, j:j + 1])
                        else:
                            evac(eng_ev, dst[0:64, j, :], pj[0:64, :],
                                 b_s[0:64, j:j + 1])
                            evac(eng_od, odd_sv[64:128, j, :], pj[64:128, :],
                                 b_s[64:128, j:j + 1])
                    # partition-shifting SBUF->SBUF DMAs for slots 8..16
                    # (emitted right after this tensor's evacs so the drain
                    # overlaps the next tensor's projection)
                    if tmaj:
                        for c in range(4):
                            cs = slice(64 * c, 64 * c + 64)
                            nc.sync.dma_start(
                                out=dst[0:64, cs, 8:16],
                                in_=dst[64:128, cs, 0:8])
                    else:
                        nc.sync.dma_start(
                            out=dst[0:64, 8:16, :], in_=odd_sv[64:128, :, :])
                # duplicate V rows (incl. ones slot) to partitions 64:128
                nc.sync.dma_start(out=vt_s[64:128, :, :], in_=vt_s[0:64, :, :])

                # next quarter's x prefetch + (q0) wo: emitted AFTER the
                # shift DMAs so their bulk descriptors don't delay the
                # shift drain that gates this quarter's attention; still a
                # full quarter ahead of their consumers
                if qq + 1 < NQ:
                    load_x("xq", xq_d, qq + 1)
                    load_x("xk", xk_d, qq + 1)
                    load_x("xv", xv_d, qq + 1)
                if qq == 0:
                    nc.sync.dma_start(out=wo_s[:], in_=wo_d[:])
                    nc.sync.dma_start(
                        out=bo_s[:], in_=bo_d.rearrange("j p -> p j"))



                # ---------- attention ----------
                for b in range(NB):          # 8 batches x 8 groups x 4 tokens
                    qpk_ps = ps.tile([128, 512], b16, tag="ps", name="qpk_ps")
                    kpk_ps = ps.tile([128, 512], b16, tag="ps", name="kpk_ps")
                    # all Q transposes BEFORE the K ones: K's shift chunks
                    # land later than Q's, so the Q batch executes while the
                    # K shift DMA is still draining
                    for src, pdst in ((qt_s, qpk_ps), (kt_s, kpk_ps)):
                        for gi in range(8):
                            g = 8 * b + gi
                            in_ = src[0:64, 4 * g:4 * g + 4, :]  # [64, 4, 32]
                            nc.tensor.transpose(
                                pdst[:, 64 * gi:64 * gi + 64], in_,
                                ident[0:64, 0:64])
                    qpk = work.tile([128, 512], b16, tag="qpk")
                    kpk = work.tile([128, 512], b16, tag="kpk")
                    nc.vector.tensor_copy(qpk[:], qpk_ps[:])
                    nc.vector.tensor_copy(kpk[:], kpk_ps[:])

                    # each token tau gets a unique (partition-half, PSUM bank):
                    # concurrent matmul drains/clears into the same bank+rows
                    # are a hardware race (observed fatal on device)
                    et_b = [ps.tile([128, 512], f32, tag="ps", name="et0"),
                            ps.tile([128, 512], f32, tag="ps", name="et1")]
                    for gi in range(8):
                        for tau in range(4):
                            nc.tensor.matmul(
                                et_b[tau // 2][64 * (tau % 2):64 * (tau % 2) + 64,
                                               64 * gi:64 * gi + 64],
                                kpk[32 * tau:32 * tau + 32,
                                    64 * gi:64 * gi + 64],
                                qpk[32 * tau:32 * tau + 32,
                                    64 * gi:64 * gi + 64],
                                start=True, stop=True,
                                tile_position=(32 * tau, 64 * (tau % 2)))
                    e_b = [epool.tile([128, 512], b16, tag="e0", name="e0"),
                           epool.tile([128, 512], b16, tag="e1", name="e1")]
                    nc.scalar.activation(e_b[0][:], et_b[0][:],
                                         mybir.ActivationFunctionType.Exp)
                    nc.scalar.activation(e_b[1][:], et_b[1][:],
                                         mybir.ActivationFunctionType.Exp)
                    pa_b = [ps.tile([128, 8, 17], f32, tag="ps", name="pa0"),
                            ps.tile([128, 8, 17], f32, tag="ps", name="pa1")]
                    for gi in range(8):
                        for tau in range(4):
                            t = 32 * b + 4 * gi + tau
                            par = tau % 2
                            nc.tensor.matmul(
                                pa_b[tau // 2][64 * par:64 * par + 64, gi, :],
                                e_b[tau // 2][64 * par:64 * par + 64,
                                              64 * gi:64 * gi + 64],
                                vt_s[64 * par:64 * par + 64, :, t],
                                start=True, stop=True)
                    # tp = 16b + 2gi + tau//2 -> even/odd interleave per bank
                    nc.vector.tensor_copy(
                        a_st[:, 16 * b:16 * b + 16:2, :], pa_b[0][:])
                    nc.vector.tensor_copy(
                        a_st[:, 16 * b + 1:16 * b + 16:2, :], pa_b[1][:])

                    # previous quarter's O-projection m-block: PE filler for
                    # this quarter's next batch's shift-chunk wait
                    if qq > 0:
                        oproj_block(qq - 1, b)

                # ---------- normalize (h-major into the all-quarter a_nm) ----
                nc.vector.reciprocal(zr_s[:], a_st[:, :, 16])
                for h in range(16):
                    nc.gpsimd.tensor_mul(
                        a_nm[:, h, qq * TP:(qq + 1) * TP],
                        a_st[:, :, h], zr_s[:, :])

            # last quarter's O-projection (the only non-overlapped one)
            for m in range(8):
                oproj_block(NQ - 1, m)
    nc.compile()
    return nc


def host_prep(q, k, v, w_q, b_q, w_k, b_k, w_v, b_v, w_o, b_o):
    j = np.arange(8)[:, None, None]
    hb = np.arange(2)[None, :, None]
    d = np.arange(64)[None, None, :]
    perm = (d * 16 + 2 * j + hb).reshape(-1)

    def prep_w(w, scale=1.0):
        wt = (w[perm, :].T.astype(np.float32) * scale).astype(bf16)
        return np.ascontiguousarray(wt.reshape(8, 128, 1024))

    com = dict(
        wq=prep_w(w_q, 0.125), wk=prep_w(w_k), wv=prep_w(w_v),
        bq=np.ascontiguousarray((b_q[perm] * 0.125).reshape(8, 128)).astype(np.float32),
        bk=np.ascontiguousarray(b_k[perm].reshape(8, 128)).astype(np.float32),
        bv=np.ascontiguousarray(b_v[perm].reshape(8, 128)).astype(np.float32),
        bo=np.ascontiguousarray(b_o.reshape(8, 128)).astype(np.float32),
    )
    # V slot order: slot j = h 2j (j<8), slot 8+j = h 2j+1
    hmap = np.array([2 * j for j in range(8)] + [2 * j + 1 for j in range(8)])
    wo_half = np.transpose(w_o.reshape(1024, 64, 16), (1, 2, 0))[:, hmap, :]
    com["wo"] = np.ascontiguousarray(
        np.concatenate([wo_half, wo_half], axis=0).astype(bf16))

    in_maps = []
    for c in range(NCORE):
        m = dict(com)
        for name, x in (("xq", q), ("xk", k), ("xv", v)):
            sl = x.reshape(-1, D)[c * T:(c + 1) * T, :]
            m[name] = np.ascontiguousarray(sl.T.astype(bf16).reshape(8, 128, T))
        in_maps.append(m)
    return in_maps


def reassemble(results):
    # per-core out [8, 2, 128, 512] -> [B, S, D]
    full = np.empty((NCORE, T, D), np.float32)
    for c, res in enumerate(results):
        od = res["out"]                     # [m=8, par=2, p=128, col=512]
        # col = qq*TP + tp ; token t = qq*T4 + 2*tp + par ; D = m*128 + p
        o = np.transpose(od, (3, 1, 0, 2))  # [col, par, m, p]
        full[c] = o.reshape(4, TP, 2, D).reshape(4, T4, D).reshape(T, D)
    return full.reshape(B, S, D)


def kernel(**inputs):
    from concourse.bass_utils import run_bass_kernel_spmd
    if "nc" not in _NC_CACHE:
        _NC_CACHE["nc"] = build_nc()
    nc = _NC_CACHE["nc"]
    in_maps = host_prep(**inputs)
    r = run_bass_kernel_spmd(nc, in_maps, core_ids=list(range(NCORE)))
    return reassemble(r.results)


if __name__ == "__main__":
    z = np.load("/root/problem/inputs_cache.npz")
    inputs = {kk: z[kk] for kk in z.files}
    expd = np.load("/root/problem/expected64.npy")
    act = kernel(**inputs)
    err = np.abs(act - expd)
    scale = np.abs(expd).max()
    print("absmax err:", err.max(), "rel:", err.max() / scale)



# revision 62
# speedup vs baseline: 1.2063x; 1.0168x over previous
"""Trainium2 Bass kernel for nn_MultiHeadAttention_59158879535767.

Reference semantics (B=4, S=2048, D=1024, H=16, DK=64):
  Q = q @ w_q.T + b_q  (same for K, V), reshaped (B,S,DK,H);
  score contracts over the HEAD axis per token: score[t] = Q_t @ K_t.T / 8
  (64x64 per token), softmax over last axis, attn[t] = score @ V_t -> (64,16),
  flattened, then @ w_o.T + b_o.

Everything is per-token => data-parallel over the 8192 tokens across 8 cores,
no collectives. Per core T=1024 tokens, processed in NQ=4 quarters of T4=256.

On-device dataflow per core (all matmuls bf16 with fp32 PSUM accumulation):
  * QKV projections: lhsT = host-permuted W.T tiles [din, (j, hb, d)] where
    output column j*128 + hb*64 + d holds dout = d*16 + (2j+hb). rhs = host-
    transposed x.T [din, tok]. PSUM [128=(hb,d), T4]; bias folded at evac.
    Q/K evac: ONE full-width [128, 128] op per (j, t-half) (rows 0:64 land
    at slot j; rows 64:128 stage at slot j of partitions 64:128 of the same
    tile - uniform AP); per-32-token-chunk SBUF->SBUF DMAs then shift the
    staged odd half down to slots 8:16 (engines cannot move partitions).
    Evac halves alternate Scalar/Vector; weight loads are staggered
    (first j-blocks land early) and x is prefetched one quarter ahead so
    its descriptors beat the shift-DMA flood into the queues.
  * Layouts: QT/KT [128, T4, 32 s] (s = h slot; rows 0:64 data, 16..31
    zeroed; rows 64:128 odd-h staging), VT [128, 17, T4] (slot 16 = ones,
    rows 64:128 duplicate of 0:64 via DMA).
  * S1 per 4-token group: one PE transpose each of QT/KT [64, (tau,s)=128]
    -> pack [128=(tau,s), 64] in PSUM (8 groups batched per bank), evac to
    SBUF. Then per token tau: matmul K=32 rows at base 32*tau:
      lhsT=Kpk[32t:+32, 64g:+64], rhs=Qpk[...] -> ET [64 e, 64 d] at
      (64*(tau%2) partitions, 64*(tau//2) free) of a [128,128] PSUM quarter.
  * exp via ACT on [128, 512] (4 groups) -> E bf16. No max subtraction needed
    (|score| <= ~3 for this distribution).
  * S2 per token: lhsT = E-slice [64 e, 64 d], rhs = VT_eh1d[64q:+64, :, t]
    [64, 17] -> out [64 d, 17] (slot 16 = sum of exp = softmax denominator).
  * normalize: A_norm = A[:, :, 0:16] * recip(A[:, :, 16]) -> bf16, written
    h-major into the all-quarter a_nm [128, 16 h, 512 t] (muls on GpSimd).
  * O-projection: deferred per-(quarter, m) blocks; per h one even-parity
    matmul on PE rows 0:64 and one odd-parity on rows 64:128 (wo rows
    64:128 duplicate 0:64) so every LDWEIGHTS hides under the other
    parity's stream; blocks of quarter q-1 are interleaved between quarter
    q's attention batches as PE stall-filler; + b_o at evac -> out DRAM
    [8 m, 2 par, 128, 512] bf16, host reassembles.
"""
import numpy as np
import ml_dtypes

B, S, D, H, DK = 4, 2048, 1024, 16, 64
NCORE = 8
T = (B * S) // NCORE          # 1024 tokens per core
NQ = 4
T4 = T // NQ                  # 256 tokens per quarter
TP = T4 // 2                  # 128 tokens per parity per quarter
NB = T4 // 32                 # 8 batches of 8 groups (32 tokens) per quarter

bf16 = ml_dtypes.bfloat16

_NC_CACHE = {}


def build_nc():
    import concourse.bacc as bacc
    import concourse.mybir as mybir
    import concourse.tile as tile
    from concourse.masks import make_identity

    nc = bacc.Bacc()
    dt = mybir.dt
    f32, b16 = dt.float32, dt.bfloat16

    # ---- DRAM I/O ----
    xq_d = nc.dram_tensor("xq", [8, 128, T], b16, kind="ExternalInput")
    xk_d = nc.dram_tensor("xk", [8, 128, T], b16, kind="ExternalInput")
    xv_d = nc.dram_tensor("xv", [8, 128, T], b16, kind="ExternalInput")
    wq_d = nc.dram_tensor("wq", [8, 128, 1024], b16, kind="ExternalInput")
    wk_d = nc.dram_tensor("wk", [8, 128, 1024], b16, kind="ExternalInput")
    wv_d = nc.dram_tensor("wv", [8, 128, 1024], b16, kind="ExternalInput")
    wo_d = nc.dram_tensor("wo", [128, 16, 1024], b16, kind="ExternalInput")
    bq_d = nc.dram_tensor("bq", [8, 128], f32, kind="ExternalInput")
    bk_d = nc.dram_tensor("bk", [8, 128], f32, kind="ExternalInput")
    bv_d = nc.dram_tensor("bv", [8, 128], f32, kind="ExternalInput")
    bo_d = nc.dram_tensor("bo", [8, 128], f32, kind="ExternalInput")
    out_d = nc.dram_tensor("out", [8, 2, 128, 4 * TP], b16, kind="ExternalOutput")

    with tile.TileContext(nc) as tc:
        with (
            tc.tile_pool(name="const", bufs=1) as const,
            tc.tile_pool(name="xin", bufs=2) as xin,
            tc.tile_pool(name="work", bufs=3) as work,
            tc.tile_pool(name="epool", bufs=3) as epool,
            tc.tile_pool(name="outp", bufs=3) as outp,
            tc.tile_pool(name="ps", bufs=6, space="PSUM") as ps,
            tc.tile_pool(name="pso", bufs=2, space="PSUM") as pso,
        ):
            # ---- persistent SBUF ----
            wq_s = const.tile([128, 8, 1024], b16, tag="wq")
            wk_s = const.tile([128, 8, 1024], b16, tag="wk")
            wv_s = const.tile([128, 8, 1024], b16, tag="wv")
            wo_s = const.tile([128, 16, 1024], b16, tag="wo")
            bq_s = const.tile([128, 8], f32, tag="bq")
            bk_s = const.tile([128, 8], f32, tag="bk")
            bv_s = const.tile([128, 8], f32, tag="bv")
            bo_s = const.tile([128, 8], f32, tag="bo")
            ident = const.tile([128, 128], b16, tag="ident")
            make_identity(nc, ident)

            # x double-buffers (explicit, so quarter-0 loads can interleave
            # with the weight loads: Q-proj work starts as early as possible)
            xt = {}
            for nm in ("xq", "xk", "xv"):
                for pb in range(2):
                    xt[nm, pb] = xin.tile([128, 8, T4], b16,
                                          tag=f"{nm}{pb}", name=f"{nm}{pb}")

            def load_x(nm, xd, qq):
                tsl = slice(qq * T4, (qq + 1) * T4)
                nc.sync.dma_start(
                    out=xt[nm, qq % 2][:],
                    in_=xd[:, :, tsl].rearrange("ko p t -> p ko t"))

            # staggered weight loads: each tensor's first j-blocks (m 0:256)
            # land early so its projection starts while the rest streams
            load_x("xq", xq_d, 0)
            wq_r = wq_d.rearrange("ko p m -> p ko m")
            wk_r = wk_d.rearrange("ko p m -> p ko m")
            wv_r = wv_d.rearrange("ko p m -> p ko m")
            nc.sync.dma_start(out=wq_s[:, :, 0:256], in_=wq_r[:, :, 0:256])
            nc.sync.dma_start(out=bq_s[:], in_=bq_d.rearrange("j p -> p j"))
            nc.sync.dma_start(out=wq_s[:, :, 256:1024], in_=wq_r[:, :, 256:1024])
            load_x("xk", xk_d, 0)
            nc.sync.dma_start(out=wk_s[:, :, 0:256], in_=wk_r[:, :, 0:256])
            nc.sync.dma_start(out=bk_s[:], in_=bk_d.rearrange("j p -> p j"))
            nc.sync.dma_start(out=wk_s[:, :, 256:1024], in_=wk_r[:, :, 256:1024])
            load_x("xv", xv_d, 0)
            nc.sync.dma_start(out=wv_s[:, :, 0:256], in_=wv_r[:, :, 0:256])
            nc.sync.dma_start(out=bv_s[:], in_=bv_d.rearrange("j p -> p j"))
            nc.sync.dma_start(out=wv_s[:, :, 256:1024], in_=wv_r[:, :, 256:1024])
            # wo/bo are first needed by oproj (mid-quarter-2); loaded inside
            # quarter 0's body so the startup burst isn't bandwidth-starved

            # token-major [64, T4, 32] so the pack-transpose weights AP is
            # contiguous (BIR requires a collapsible stationary AP)
            qt_s = const.tile([128, T4, 32], b16, tag="qt")   # rows 0:64 used
            kt_s = const.tile([128, T4, 32], b16, tag="kt")
            vt_s = const.tile([128, 17, T4], b16, tag="vt")
            # odd-h staging for V (slot-major, rows 64:128 used); Q/K stage
            # their odd half inside qt_s/kt_s rows 64:128 at slot j
            odd_sv = const.tile([128, 8, T4], b16, tag="oddv")
            a_st = const.tile([128, TP, 17], b16, tag="ast")
            zr_s = const.tile([128, TP], f32, tag="zr")
            # h-major, all 4 quarters: col = qq*TP + tp, partition = (par, d)
            a_nm = const.tile([128, 16, 4 * TP], b16, tag="anorm")

            # zero pad slots (s = 16..32) of QT/KT once; ones slot for V once
            nc.any.memset(qt_s[0:64, :, 16:32], 0.0)
            nc.any.memset(kt_s[0:64, :, 16:32], 0.0)
            nc.any.memset(vt_s[0:64, 16, :], 1.0)

            def evac(eng, dst, src, bias):
                if eng == "v":
                    nc.vector.tensor_scalar_add(dst, src, bias)
                else:
                    nc.scalar.activation(
                        dst, src, mybir.ActivationFunctionType.Identity,
                        bias=bias, scale=1.0)

            def oproj_block(qx, m):
                # O-projection m-block over quarter qx (a_nm cols qx*TP..).
                # per h: even-parity matmul on PE rows 0:64, odd on rows
                # 64:128 (wo_s rows 64:128 duplicate 0:64) -> alternating
                # row-groups let each LDWEIGHTS hide under the other matmul's
                # stream.  Own 2-bank PSUM pool so attention batches keep
                # their 6 banks.  One m-block is emitted between consecutive
                # attention batches: the PE chews it while a batch waits for
                # its slot-shift DMA chunk.
                csl = slice(qx * TP, (qx + 1) * TP)
                po_e = pso.tile([128, TP], f32, tag="pso", name="poe")
                po_o = pso.tile([128, TP], f32, tag="pso", name="poo")
                for h in range(16):
                    nc.tensor.matmul(
                        po_e, wo_s[0:64, h, m * 128:(m + 1) * 128],
                        a_nm[0:64, h, csl],
                        start=(h == 0), stop=(h == 15))
                    nc.tensor.matmul(
                        po_o, wo_s[64:128, h, m * 128:(m + 1) * 128],
                        a_nm[64:128, h, csl],
                        start=(h == 0), stop=(h == 15))
                o_e = outp.tile([128, TP], b16, tag="o", name="oe")
                o_o = outp.tile([128, TP], b16, tag="o", name="oo")
                nc.scalar.activation(
                    o_e[:], po_e[:], mybir.ActivationFunctionType.Identity,
                    bias=bo_s[:, m:m + 1], scale=1.0)
                nc.scalar.activation(
                    o_o[:], po_o[:], mybir.ActivationFunctionType.Identity,
                    bias=bo_s[:, m:m + 1], scale=1.0)
                nc.sync.dma_start(out=out_d[m, 0, :, csl], in_=o_e[:])
                nc.sync.dma_start(out=out_d[m, 1, :, csl], in_=o_o[:])

            for qq in range(NQ):
                # ---------- projections ----------
                # evac engines: each tensor's even/odd halves go to different
                # engines so Scalar+Vector chew each j-block concurrently;
                # odd-slot partition-shift DMA issued per-j so it pipelines.
                for x_t, w_s, b_s, dst, tmaj, eng_ev, eng_od in (
                    (xt["xq", qq % 2], wq_s, bq_s, qt_s, True, "v", "s"),
                    (xt["xk", qq % 2], wk_s, bk_s, kt_s, True, "s", "v"),
                    (xt["xv", qq % 2], wv_s, bv_s, vt_s, False, "s", "v"),
                ):
                    for j in range(8):
                        pj = ps.tile([128, 512], f32, tag="ps", name="pj")[:, :T4]
                        for ko in range(8):
                            nc.tensor.matmul(
                                pj, w_s[:, ko, j * 128:(j + 1) * 128],
                                x_t[:, ko, :],
                                start=(ko == 0), stop=(ko == 7))
                        if tmaj:
                            # full-width evacs in two t-halves: rows 0:64
                            # (h=2j) land at slot j directly; rows 64:128
                            # (h=2j+1) stage at slot j of partitions 64:128
                            # (uniform AP); shift DMAs fold them to slots
                            # 8:16 in t-chunks so attention batch 0 only
                            # waits for chunk 0
                            evac(eng_ev, dst[:, 0:128, j], pj[:, 0:128],
                                 b_s[:, j:j + 1])
                            evac(eng_od, dst[:, 128:256, j], pj[:, 128:256],
                                 b_s[:, j:j + 1])
                        else:
                            evac(eng_ev, dst[0:64, j, :], pj[0:64, :],
                                 b_s[0:64, j:j + 1])
                            evac(eng_od, odd_sv[64:128, j, :], pj[64:128, :],
                                 b_s[64:128, j:j + 1])
                    # partition-shifting SBUF->SBUF DMAs for slots 8..16
                    # (emitted right after this tensor's evacs so the drain
                    # overlaps the next tensor's projection)
                    if tmaj:
                        for c in range(4):
                            cs = slice(64 * c, 64 * c + 64)
                            nc.sync.dma_start(
                                out=dst[0:64, cs, 8:16],
                                in_=dst[64:128, cs, 0:8])
                    else:
                        nc.sync.dma_start(
                            out=dst[0:64, 8:16, :], in_=odd_sv[64:128, :, :])
                # duplicate V rows (incl. ones slot) to partitions 64:128
                nc.sync.dma_start(out=vt_s[64:128, :, :], in_=vt_s[0:64, :, :])

                # next quarter's x prefetch + (q0) wo: emitted AFTER the
                # shift DMAs so their bulk descriptors don't delay the
                # shift drain that gates this quarter's attention; still a
                # full quarter ahead of their consumers
                if qq + 1 < NQ:
                    load_x("xq", xq_d, qq + 1)
                    load_x("xk", xk_d, qq + 1)
                    load_x("xv", xv_d, qq + 1)
                if qq == 0:
                    nc.sync.dma_start(out=wo_s[:], in_=wo_d[:])
                    nc.sync.dma_start(
                        out=bo_s[:], in_=bo_d.rearrange("j p -> p j"))



                # ---------- attention ----------
                for b in range(NB):          # 8 batches x 8 groups x 4 tokens
                    # previous quarter's O-projection m-block FIRST: the PE
                    # chews it while this batch's shift-DMA chunks drain
                    # (most critically in front of batch 0's K wait)
                    if qq > 0:
                        oproj_block(qq - 1, b)
                    qpk_ps = ps.tile([128, 512], b16, tag="ps", name="qpk_ps")
                    kpk_ps = ps.tile([128, 512], b16, tag="ps", name="kpk_ps")
                    # all Q transposes BEFORE the K ones: K's shift chunks
                    # land later than Q's, so the Q batch executes while the
                    # K shift DMA is still draining
                    for src, pdst in ((qt_s, qpk_ps), (kt_s, kpk_ps)):
                        for gi in range(8):
                            g = 8 * b + gi
                            in_ = src[0:64, 4 * g:4 * g + 4, :]  # [64, 4, 32]
                            nc.tensor.transpose(
                                pdst[:, 64 * gi:64 * gi + 64], in_,
                                ident[0:64, 0:64])
                    qpk = work.tile([128, 512], b16, tag="qpk")
                    kpk = work.tile([128, 512], b16, tag="kpk")
                    nc.vector.tensor_copy(qpk[:], qpk_ps[:])
                    nc.vector.tensor_copy(kpk[:], kpk_ps[:])

                    # each token tau gets a unique (partition-half, PSUM bank):
                    # concurrent matmul drains/clears into the same bank+rows
                    # are a hardware race (observed fatal on device)
                    et_b = [ps.tile([128, 512], f32, tag="ps", name="et0"),
                            ps.tile([128, 512], f32, tag="ps", name="et1")]
                    for gi in range(8):
                        for tau in range(4):
                            nc.tensor.matmul(
                                et_b[tau // 2][64 * (tau % 2):64 * (tau % 2) + 64,
                                               64 * gi:64 * gi + 64],
                                kpk[32 * tau:32 * tau + 32,
                                    64 * gi:64 * gi + 64],
                                qpk[32 * tau:32 * tau + 32,
                                    64 * gi:64 * gi + 64],
                                start=True, stop=True,
                                tile_position=(32 * tau, 64 * (tau % 2)))
                    e_b = [epool.tile([128, 512], b16, tag="e0", name="e0"),
                           epool.tile([128, 512], b16, tag="e1", name="e1")]
                    nc.scalar.activation(e_b[0][:], et_b[0][:],
                                         mybir.ActivationFunctionType.Exp)
                    nc.scalar.activation(e_b[1][:], et_b[1][:],
                                         mybir.ActivationFunctionType.Exp)
                    pa_b = [ps.tile([128, 8, 17], f32, tag="ps", name="pa0"),
                            ps.tile([128, 8, 17], f32, tag="ps", name="pa1")]
                    for gi in range(8):
                        for tau in range(4):
                            t = 32 * b + 4 * gi + tau
                            par = tau % 2
                            nc.tensor.matmul(
                                pa_b[tau // 2][64 * par:64 * par + 64, gi, :],
                                e_b[tau // 2][64 * par:64 * par + 64,
                                              64 * gi:64 * gi + 64],
                                vt_s[64 * par:64 * par + 64, :, t],
                                start=True, stop=True)
                    # tp = 16b + 2gi + tau//2 -> even/odd interleave per bank
                    nc.vector.tensor_copy(
                        a_st[:, 16 * b:16 * b + 16:2, :], pa_b[0][:])
                    nc.vector.tensor_copy(
                        a_st[:, 16 * b + 1:16 * b + 16:2, :], pa_b[1][:])

                # ---------- normalize (h-major into the all-quarter a_nm) ----
                nc.vector.reciprocal(zr_s[:], a_st[:, :, 16])
                for h in range(16):
                    nc.gpsimd.tensor_mul(
                        a_nm[:, h, qq * TP:(qq + 1) * TP],
                        a_st[:, :, h], zr_s[:, :])

            # last quarter's O-projection (the only non-overlapped one)
            for m in range(8):
                oproj_block(NQ - 1, m)
    nc.compile()
    return nc


def host_prep(q, k, v, w_q, b_q, w_k, b_k, w_v, b_v, w_o, b_o):
    j = np.arange(8)[:, None, None]
    hb = np.arange(2)[None, :, None]
    d = np.arange(64)[None, None, :]
    perm = (d * 16 + 2 * j + hb).reshape(-1)

    def prep_w(w, scale=1.0):
        wt = (w[perm, :].T.astype(np.float32) * scale).astype(bf16)
        return np.ascontiguousarray(wt.reshape(8, 128, 1024))

    com = dict(
        wq=prep_w(w_q, 0.125), wk=prep_w(w_k), wv=prep_w(w_v),
        bq=np.ascontiguousarray((b_q[perm] * 0.125).reshape(8, 128)).astype(np.float32),
        bk=np.ascontiguousarray(b_k[perm].reshape(8, 128)).astype(np.float32),
        bv=np.ascontiguousarray(b_v[perm].reshape(8, 128)).astype(np.float32),
        bo=np.ascontiguousarray(b_o.reshape(8, 128)).astype(np.float32),
    )
    # V slot order: slot j = h 2j (j<8), slot 8+j = h 2j+1
    hmap = np.array([2 * j for j in range(8)] + [2 * j + 1 for j in range(8)])
    wo_half = np.transpose(w_o.reshape(1024, 64, 16), (1, 2, 0))[:, hmap, :]
    com["wo"] = np.ascontiguousarray(
        np.concatenate([wo_half, wo_half], axis=0).astype(bf16))

    in_maps = []
    for c in range(NCORE):
        m = dict(com)
        for name, x in (("xq", q), ("xk", k), ("xv", v)):
            sl = x.reshape(-1, D)[c * T:(c + 1) * T, :]
            m[name] = np.ascontiguousarray(sl.T.astype(bf16).reshape(8, 128, T))
        in_maps.append(m)
    return in_maps


def reassemble(results):
    # per-core out [8, 2, 128, 512] -> [B, S, D]
    full = np.empty((NCORE, T, D), np.float32)
    for c, res in enumerate(results):
        od = res["out"]                     # [m=8, par=2, p=128, col=512]
        # col = qq*TP + tp ; token t = qq*T4 + 2*tp + par ; D = m*128 + p
        o = np.transpose(od, (3, 1, 0, 2))  # [col, par, m, p]
        full[c] = o.reshape(4, TP, 2, D).reshape(4, T4, D).reshape(T, D)
    return full.reshape(B, S, D)


def kernel(**inputs):
    from concourse.bass_utils import run_bass_kernel_spmd
    if "nc" not in _NC_CACHE:
        _NC_CACHE["nc"] = build_nc()
    nc = _NC_CACHE["nc"]
    in_maps = host_prep(**inputs)
    r = run_bass_kernel_spmd(nc, in_maps, core_ids=list(range(NCORE)))
    return reassemble(r.results)


if __name__ == "__main__":
    z = np.load("/root/problem/inputs_cache.npz")
    inputs = {kk: z[kk] for kk in z.files}
    expd = np.load("/root/problem/expected64.npy")
    act = kernel(**inputs)
    err = np.abs(act - expd)
    scale = np.abs(expd).max()
    print("absmax err:", err.max(), "rel:", err.max() / scale)

